# revision 1
# baseline (speedup 1.0000x reference)
"""Multi-head causal self-attention (V=Q variant) on 8 Trainium2 cores.

Sharding: batch (2) x head-group (4 groups of 4 heads). Each core computes
full-sequence attention for its 4 heads of one batch element, plus its slice
of the output projection; the host sums the 4 partial projections per batch
and adds b0.

Per core: xT [1024, 2048], Wq_s/Wk_s [1024, 256], W0_s [256, 1024].

Scores are computed transposed (S^T[kv, q]) so the softmax denominator falls
out of the AV matmul via a ones-column appended to V (V aliases Q in this
module -- the reference replicates that bug). The 1/sqrt(DK) scale is folded
into Wk/bk on the host. All matmul inputs live in float32r SBUF tiles (TF32-
like, 1 cycle/row on the PE vs 4 for fp32); PSUM accumulation stays fp32.

ACT (exp) is the long-pole engine, so program order interleaves projection
sub-sweeps with attention j-blocks to start the exp stream as early as
possible. DVE does PSUM evictions + normalize, Pool does masks/broadcasts.
Causal diagonal tiles are narrowed to their valid q-range (left-trimmed).
"""

import ml_dtypes
import numpy as np

import concourse.bacc as bacc
import concourse.mybir as mybir
from concourse.tile import TileContext, add_dep_helper

P = 128
S = 2048  # sequence length
D = 1024  # model dim
HD = 256  # head-group width (4 heads x 64)
DK = 64
NQ = 4  # q chunks of 512
NKV = 16  # kv chunks of 128
NKD = 8  # D chunks of 128
F32 = mybir.dt.float32
F32R = mybir.dt.float32r
BF16 = mybir.dt.bfloat16
EXP = mybir.ActivationFunctionType.Exp

_CACHED_NC = None


def build_nc():
    nc = bacc.Bacc("TRN2", target_bir_lowering=False, debug=False, num_devices=8)
    xT = nc.declare_dram_parameter("xT", [D, S], BF16, isOutput=False)
    Wq = nc.declare_dram_parameter("Wq", [D, HD], BF16, isOutput=False)
    Wk = nc.declare_dram_parameter("Wk", [D, HD], BF16, isOutput=False)
    bqt = nc.declare_dram_parameter("bqt", [P, 2], F32, isOutput=False)
    bkt = nc.declare_dram_parameter("bkt", [P, 2], F32, isOutput=False)
    W0 = nc.declare_dram_parameter("W0", [HD, D], F32, isOutput=False)
    out = nc.declare_dram_parameter("out", [S, D], BF16, isOutput=True)

    def rd(ap):
        # reinterpret a DRAM f32 region as f32r for raw DMA into f32r tiles
        return ap.bitcast(F32R)

    with TileContext(nc) as tc:
        with (
            tc.tile_pool(name="const", bufs=1) as const,
            tc.tile_pool(name="xt", bufs=16) as xtp,
            tc.tile_pool(name="wqk", bufs=1) as wp,
            tc.tile_pool(name="vp", bufs=32) as vpool,
            tc.tile_pool(name="pt", bufs=6) as ptp,
            tc.tile_pool(name="dp", bufs=4) as dpool,
            tc.tile_pool(name="ost", bufs=3) as ostp,
            tc.tile_pool(name="mm", bufs=2, space="PSUM") as mmp,
            tc.tile_pool(name="sps", bufs=2, space="PSUM") as spsum,
            tc.tile_pool(name="aps", bufs=2, space="PSUM") as apsum,
        ):
            identity = const.tile([P, P], F32)
            nc.gpsimd.memset(identity[:], 0.0)
            nc.gpsimd.affine_select(
                out=identity[:],
                in_=identity[:],
                compare_op=mybir.AluOpType.not_equal,
                fill=1.0,
                base=0,
                pattern=[[-1, P]],
                channel_multiplier=1,
            )
            ones_col = const.tile([P, 1], F32)
            nc.gpsimd.memset(ones_col[:], 1.0)
            # triangular mask [128,128]: keep (1.0) where q >= kv, i.e. f >= p
            tri = const.tile([P, P], F32, name="tri")
            nc.gpsimd.memset(tri[:], 1.0)
            nc.gpsimd.affine_select(
                out=tri[:],
                in_=tri[:],
                compare_op=mybir.AluOpType.is_ge,
                fill=0.0,
                base=0,
                pattern=[[1, P]],
                channel_multiplier=-1,
            )
            # [128,256] mask: zeros block then triangle (for left-padded tiles)
            ztri = const.tile([P, 256], F32, name="ztri")
            nc.gpsimd.memset(ztri[:], 1.0)
            nc.gpsimd.affine_select(
                out=ztri[:],
                in_=ztri[:],
                compare_op=mybir.AluOpType.is_ge,
                fill=0.0,
                base=-128,
                pattern=[[1, 256]],
                channel_multiplier=-1,
            )
            # ACT exp-table warmup while DMAs run
            warm = const.tile([P, 8], F32, name="warm")
            nc.gpsimd.memset(warm[:], 0.0)
            nc.scalar.activation(out=warm[:], in_=warm[:], func=EXP)

            bq_sb = const.tile([P, 2], F32)
            bk_sb = const.tile([P, 2], F32)
            w0_sb = [const.tile([P, D], F32R, name=f"w0_{kc}") for kc in range(2)]
            # QT/KT as [mi][ni] tiles of [128, 512] for fine-grained deps
            QT = [
                [const.tile([P, 512], F32R, name=f"qt{mi}_{ni}") for ni in range(NQ)]
                for mi in range(2)
            ]
            KT = [
                [const.tile([P, 512], F32R, name=f"kt{mi}_{ni}") for ni in range(NQ)]
                for mi in range(2)
            ]
            # normalized attention (transposed), per q-chunk and head-pair
            attn = [
                [const.tile([P, 512], F32R, name=f"attn{j}_{p}") for p in range(2)]
                for j in range(4)
            ]

            # weights in one strided DMA each (per-DMA issue cadence is ~650ns,
            # so many small DMAs would serialize the stream)
            wq_big = wp.tile([P, NKD, HD], BF16, name="wqb")
            nc.sync.dma_start(out=wq_big[:], in_=Wq.rearrange("(k p) c -> p k c", p=P))
            wk_big = wp.tile([P, NKD, HD], BF16, name="wkb")
            nc.sync.dma_start(out=wk_big[:], in_=Wk.rearrange("(k p) c -> p k c", p=P))
            wq_t = [wq_big[:, k, :] for k in range(NKD)]
            wk_t = [wk_big[:, k, :] for k in range(NKD)]
            nc.sync.dma_start(out=bq_sb[:], in_=bqt[:, :])
            nc.sync.dma_start(out=bk_sb[:], in_=bkt[:, :])
            # xT as [k][half] tiles of [128, 1024]; 4 serialized chains so
            # all half-0 tiles (cols 0:1024) land before any half-1.
            xh = [
                [xtp.tile([P, 1024], BF16, name="xtile") for _ in range(2)]
                for _ in range(NKD)
            ]
            # x half-0 chunks first, then half-1, then W0: the DMA path
            # drains in issue order, which staggers arrivals naturally
            for h in range(2):
                for k in range(NKD):
                    nc.sync.dma_start(
                        out=xh[k][h][:],
                        in_=xT[k * P : (k + 1) * P, h * 1024 : (h + 1) * 1024],
                    )
            for kc in range(2):
                nc.sync.dma_start(
                    out=w0_sb[kc][:], in_=rd(W0[kc * P : (kc + 1) * P, :])
                )

            def sweep_items(ni, mi):
                """Projection sub-sweep as a list of emit-thunks (per-k)."""
                half, col = divmod(ni, 2)
                pss = [mmp.tile([P, 512], F32, name="ps") for _ in range(2)]

                def mk(k):
                    def go():
                        for ps, wt in zip(pss, (wq_t, wk_t)):
                            nc.tensor.matmul(
                                ps[:],
                                lhsT=wt[k][:, mi * P : (mi + 1) * P],
                                rhs=xh[k][half][:, col * 512 : (col + 1) * 512],
                                start=(k == 0),
                                stop=(k == NKD - 1),
                            )
                    return go

                def evict():
                    for ps, bias, dstT in zip(pss, (bq_sb, bk_sb), (QT, KT)):
                        nc.vector.tensor_scalar_add(
                            dstT[mi][ni][:, :], ps[:], bias[:, mi : mi + 1]
                        )

                return [mk(k) for k in range(NKD)] + [evict]

            vp = {}

            def emit_transposes(pair, i_lo, i_hi):
                # V' tiles [128, 132]: A data 0:64, A one 64, B data 66:130, B one 130
                for i in range(i_lo, i_hi):
                    tp = spsum.tile([P, P], F32, name="spsA")
                    nc.tensor.transpose(
                        tp[:, 0:P],
                        QT[pair][i // 4][:, (i % 4) * P : (i % 4 + 1) * P].bitcast(F32),
                        identity[:],
                    )
                    vt = vpool.tile([P, 132], BF16, name="vt")
                    nc.vector.tensor_copy(vt[:, 0:64], tp[:, 0:64])
                    nc.vector.tensor_copy(vt[:, 66:130], tp[:, 64:128])
                    nc.gpsimd.tensor_copy(vt[:, 64:65], ones_col[:])
                    nc.gpsimd.tensor_copy(vt[:, 130:131], ones_col[:])
                    vp[(pair, i)] = vt

            bg = []  # drip queue of (cost, thunk): sweeps, then phase-C blocks

            def drip(budget):
                while bg and budget > 0:
                    cost, thunk = bg.pop(0)
                    thunk()
                    budget -= cost

            def emit_cblock_m(j, c):
                m = j * 4 + c
                ot = ostp.tile([P, D], BF16, name="ot")
                for n in range(2):
                    ps = mmp.tile([P, 512], F32, name="ps")
                    for kc in range(2):
                        nc.tensor.matmul(
                            ps[:],
                            lhsT=attn[j][kc][:, c * P : (c + 1) * P],
                            rhs=w0_sb[kc][:, n * 512 : (n + 1) * 512],
                            start=(kc == 0),
                            stop=(kc == 1),
                        )
                    nc.vector.tensor_copy(ot[:, n * 512 : (n + 1) * 512], ps[:])
                nc.sync.dma_start(out=out[m * P : (m + 1) * P, :], in_=ot[:])

            def emit_pair(pair):
                steps = [(j, i) for j in range(NQ) for i in range(4 * j + 4)]
                ats = {}
                pend = None

                def emit_S(j, i):
                    off = max(0, i * P - j * 512)  # 0,128,256,384
                    w = 512 - off
                    # f32r matmuls need free>=256: pad the S matmul leftward
                    # for the 128-wide case, but exp/AV (bf16) use true width
                    swoff, swm = (256, 256) if w == P else (off, w)
                    kc = slice((i % 4) * P, (i % 4 + 1) * P)
                    sA = spsum.tile([P, 512], F32, name="spsA")
                    sB = spsum.tile([P, 512], F32, name="spsB")
                    nc.tensor.matmul(
                        sA[:, 0:swm],
                        lhsT=KT[pair][i // 4][0:64, kc],
                        rhs=QT[pair][j][0:64, swoff : swoff + swm],
                    )
                    nc.tensor.matmul(
                        sB[:, 0:swm],
                        lhsT=KT[pair][i // 4][64:128, kc],
                        rhs=QT[pair][j][64:128, swoff : swoff + swm],
                    )
                    qsl = slice(off, 512)
                    sskip = off - swoff  # valid region offset within s psum
                    pA = ptp.tile([P, 512], BF16, name="ptA")
                    pB = ptp.tile([P, 512], BF16, name="ptB")
                    nc.scalar.activation(
                        out=pA[:, 0:w], in_=sA[:, sskip : sskip + w], func=EXP
                    )
                    nc.scalar.activation(
                        out=pB[:, 0:w], in_=sB[:, sskip : sskip + w], func=EXP
                    )
                    if i >= 4 * j:  # diagonal tile: mask the leading block
                        nc.vector.tensor_mul(pA[:, 0:P], pA[:, 0:P], tri[:])
                        nc.vector.tensor_mul(pB[:, 0:P], pB[:, 0:P], tri[:])
                    return (j, i, pA, pB, qsl, w)

                def emit_AV(j, i, pA, pB, qsl, wm):
                    if i == 0:
                        ats[j] = (
                            apsum.tile([P, 512], F32, name="aps"),
                            apsum.tile([P, 512], F32, name="aps"),
                        )
                    atA, atB = ats[j]
                    imax = 4 * j + 3
                    vt = vp[(pair, i)]
                    nc.tensor.matmul(
                        atA[0:65, qsl],
                        lhsT=vt[:, 0:65],
                        rhs=pA[:, 0:wm],
                        start=(i == 0),
                        stop=(i == imax),
                    )
                    nc.tensor.matmul(
                        atB[0:65, qsl],
                        lhsT=vt[:, 66:131],
                        rhs=pB[:, 0:wm],
                        start=(i == 0),
                        stop=(i == imax),
                    )
                    if i == imax:  # normalize: attn = att_un / d, d = row 64
                        for at, rows in ((atA, slice(0, 64)), (atB, slice(64, 128))):
                            rec = dpool.tile([1, 512], F32, name="rec")
                            nc.vector.reciprocal(rec[:], at[64:65, :])
                            rbc = dpool.tile([64, 512], F32, name="rbc")
                            nc.gpsimd.partition_broadcast(rbc[0:64, :], rec[0:1, :])
                            nc.vector.tensor_mul(
                                attn[j][pair][rows, :], at[0:64, :], rbc[0:64, :]
                            )
                        if pair == 1:  # output projection becomes available
                            for c in range(4):
                                bg.append((1, lambda j=j, c=c: emit_cblock_m(j, c)))

                for j, i in steps:
                    cur = emit_S(j, i)
                    drip(5)
                    if pend is not None:
                        emit_AV(*pend)
                    pend = cur
                emit_AV(*pend)

            def t_item(pair, i):
                return (1, lambda: emit_transposes(pair, i, i + 1))

            # upfront: pair-0 ni=0 projection (DMA-paced) + first V transposes
            for it in sweep_items(0, 0):
                it()
            emit_transposes(0, 0, 4)
            # bg order follows need-by and DMA-arrival order; cost 2 paces the
            # first half-1-gated sweep to the chunk arrival rate
            for ni, mi in ((1, 0),):
                bg.extend((1, it) for it in sweep_items(ni, mi))
            bg.extend(t_item(0, i) for i in range(4, 8))
            bg.extend((1, it) for it in sweep_items(2, 0))
            bg.extend(t_item(0, i) for i in range(8, 12))
            bg.extend((1, it) for it in sweep_items(3, 0))
            bg.extend(t_item(0, i) for i in range(12, 16))
            bg.extend((1, it) for it in sweep_items(0, 1))
            bg.extend((1, it) for it in sweep_items(1, 1))
            bg.extend((1, it) for it in sweep_items(2, 1))
            bg.extend((1, it) for it in sweep_items(3, 1))
            bg.extend(t_item(1, i) for i in range(0, 16))
            emit_pair(0)
            emit_pair(1)
            while bg:
                drip(5)

    nc.compile()
    return nc


def make_in_maps(pos_encode_toks, Wq, bq, Wk, bk, W0, b0):
    x = np.asarray(pos_encode_toks, dtype=np.float32)
    Wq = np.asarray(Wq, dtype=np.float32)
    bq = np.asarray(bq, dtype=np.float32)
    Wk = np.asarray(Wk, dtype=np.float32)
    bk = np.asarray(bk, dtype=np.float32)
    W0 = np.asarray(W0, dtype=np.float32)
    in_maps = []
    for core in range(8):
        b, g = divmod(core, 4)
        hs = slice(g * HD, (g + 1) * HD)
        scale = np.float32(1.0 / np.sqrt(DK))
        in_maps.append(
            {
                "xT": np.ascontiguousarray(x[b].T).astype(ml_dtypes.bfloat16),
                "Wq": np.ascontiguousarray(Wq[:, hs]).astype(ml_dtypes.bfloat16),
                "Wk": np.ascontiguousarray(Wk[:, hs] * scale).astype(ml_dtypes.bfloat16),
                "bqt": np.ascontiguousarray(bq[hs].reshape(2, P).T),
                "bkt": np.ascontiguousarray((bk[hs] * scale).reshape(2, P).T),
                "W0": np.ascontiguousarray(W0[hs, :]),
            }
        )
    return in_maps


def assemble(results, b0):
    out = np.zeros((2, S, D), dtype=np.float32)
    for core in range(8):
        b = core // 4
        out[b] += results[core]["out"].astype(np.float32)
    out += np.asarray(b0, dtype=np.float32)
    return out


def kernel(pos_encode_toks, Wq, bq, Wk, bk, W0, b0):
    from concourse.bass_utils import run_bass_kernel_spmd

    global _CACHED_NC
    if _CACHED_NC is None:
        _CACHED_NC = build_nc()
    in_maps = make_in_maps(pos_encode_toks, Wq, bq, Wk, bk, W0, b0)
    res = run_bass_kernel_spmd(_CACHED_NC, in_maps, core_ids=list(range(8)))
    return assemble(res.results, b0)



# revision 4
# speedup vs baseline: 1.2353x; 1.2353x over previous
"""Multi-head causal self-attention (V=Q variant) on 8 Trainium2 cores, v2.

Sharding: batch (2) x head-group (4 groups of 4 heads). Each core computes
full-sequence attention for its 4 heads (2 head-pairs) of one batch element
plus its slice of the output projection; the host sums 4 partials per batch
and adds b0.

v2 redesign vs v1:
- K projection and scores run in fp8e4m3 DoubleRow (0.5 cycles/row).
  Scores use a zeroed second k-tile so a 64-deep contraction still gets the
  DoubleRow rate. Wk is host-prescaled by 128 to keep fp8 normals; the
  1/128 and the 1/sqrt(DK) are folded into the K eviction and exp scale.
- AV is computed reversed: out[q,65] = probs_chunk^T @ [V|1], filling all
  128 output partitions per pass (the forward form only filled 65). The
  softmax denominator still falls out of the ones column; normalization is
  a per-partition divide (Pool), then attn^T is rebuilt with one PE
  transpose per (pair, m) for the output projection.
- exp is merged to one [128,1024] activation per full score tile (A and B
  halves share a 2-bank PSUM tile); ACT is the bottleneck engine.
- Evictions are spread: DVE does projections/V'/attnT, Pool does normalize
  divides and output-projection evictions.
"""

import ml_dtypes
import numpy as np

import concourse.bacc as bacc
import concourse.mybir as mybir
from concourse.tile import TileContext, add_dep_helper

P = 128
S = 2048
D = 1024
HD = 256
DK = 64
NQ = 4  # q blocks of 512
NKD = 8  # D chunks of 128
F32 = mybir.dt.float32
BF16 = mybir.dt.bfloat16
FP8 = mybir.dt.float8e4
EXP = mybir.ActivationFunctionType.Exp
DR = mybir.MatmulPerfMode.DoubleRow
MUL = mybir.AluOpType.mult
ADD = mybir.AluOpType.add
DIV = mybir.AluOpType.divide

_CACHED_NC = None


def build_nc():
    nc = bacc.Bacc("TRN2", target_bir_lowering=False, debug=False, num_devices=8)
    xT = nc.declare_dram_parameter("xT", [D, S], BF16, isOutput=False)
    x8 = nc.declare_dram_parameter("x8", [D, S], FP8, isOutput=False)
    Wq = nc.declare_dram_parameter("Wq", [D, HD], BF16, isOutput=False)
    Wk8 = nc.declare_dram_parameter("Wk8", [D, HD], FP8, isOutput=False)
    bqt = nc.declare_dram_parameter("bqt", [P, 2], F32, isOutput=False)
    bkt = nc.declare_dram_parameter("bkt", [P, 2], F32, isOutput=False)
    W0 = nc.declare_dram_parameter("W0", [HD, D], BF16, isOutput=False)
    out = nc.declare_dram_parameter("out", [S, D], BF16, isOutput=True)

    with TileContext(nc) as tc:
        with (
            tc.tile_pool(name="const", bufs=1) as const,
            tc.tile_pool(name="xt", bufs=8) as xtp,
            tc.tile_pool(name="x8p", bufs=4) as x8p,
            tc.tile_pool(name="pp", bufs=24) as ppool,
            tc.tile_pool(name="an", bufs=4) as anp,
            tc.tile_pool(name="tst", bufs=3) as tstg,
            tc.tile_pool(name="rc", bufs=4) as rcp,
            tc.tile_pool(name="ost", bufs=3) as ostp,
            tc.tile_pool(name="sps", bufs=2, space="PSUM") as sps,
            tc.tile_pool(name="aps", bufs=2, space="PSUM") as aps,
            tc.tile_pool(name="mm", bufs=2, space="PSUM") as mmp,
        ):
            # ---- PE p-state warmup first: dummy matmuls keep the PE busy
            # through the initial DMA wait so real matmuls start at full clock
            wa = const.tile([P, 256], BF16, name="wa")
            nc.gpsimd.memset(wa[:], 0.0)
            for _ in range(14):
                wps = mmp.tile([P, 256], F32, name="ps")
                nc.tensor.matmul(wps[:], lhsT=wa[:, 0:128], rhs=wa[:], start=True, stop=True)

            # ---- constants / static tiles
            identity = const.tile([P, P], BF16)
            nc.gpsimd.memset(identity[:], 0.0)
            nc.gpsimd.affine_select(
                out=identity[:],
                in_=identity[:],
                compare_op=mybir.AluOpType.not_equal,
                fill=1.0,
                base=0,
                pattern=[[-1, P]],
                channel_multiplier=1,
            )
            # triangular keep-mask [128,128]: 1.0 where q >= kv
            tri = const.tile([P, P], BF16, name="tri")
            nc.gpsimd.memset(tri[:], 1.0)
            nc.gpsimd.affine_select(
                out=tri[:],
                in_=tri[:],
                compare_op=mybir.AluOpType.is_ge,
                fill=0.0,
                base=0,
                pattern=[[1, P]],
                channel_multiplier=-1,
            )
            # ACT exp-table warmup while DMAs run
            warm = const.tile([P, 8], F32, name="warm")
            nc.gpsimd.memset(warm[:], 0.0)
            nc.scalar.activation(out=warm[:], in_=warm[:], func=EXP)

            bq_sb = const.tile([P, 2], F32)
            bk_sb = const.tile([P, 2], F32)
            w0_sb = [const.tile([P, D], BF16, name=f"w0_{p}") for p in range(2)]
            # projections: QT (bf16, feeds V' transposes), QT8/KT8 (fp8;
            # scores use a stride-0 broadcast second k-tile, doubling the
            # result -- absorbed by the exp scale)
            QT = [
                [const.tile([P, 512], BF16, name=f"qt{mi}_{ni}") for ni in range(NQ)]
                for mi in range(2)
            ]
            QT8 = [
                [const.tile([P, 1, 512], FP8, name=f"qt8{mi}_{ni}") for ni in range(NQ)]
                for mi in range(2)
            ]
            KT8 = [
                [const.tile([P, 1, 512], FP8, name=f"kt8{mi}_{ni}") for ni in range(NQ)]
                for mi in range(2)
            ]
            vt = {}
            for pair in range(2):
                for i in range(16):
                    vt[(pair, i)] = const.tile([P, 2, 65], BF16, name=f"vt{pair}_{i}")
            attnT = {}
            for pair in range(2):
                for m in range(16):
                    attnT[(pair, m)] = const.tile([P, P], BF16, name=f"at{pair}_{m}")

            # ones column of every V' tile (Pool, pair-0 first); written once
            for key in vt:
                nc.gpsimd.memset(vt[key][:, :, 64:65], 1.0)

            # ---- DMAs, ordered so the first Q/K sweeps can start early.
            # x arrives in 512-column (q-chunk) slabs, k-major inside.
            xq = [[None, None] for _ in range(NQ)]  # [qc][lohi] -> [P, 4, 512]
            x8q = [None] * NQ  # [qc] -> [P, 8, 512] fp8
            wq_big = const.tile([P, NKD, HD], BF16, name="wqb")
            wk8_big = const.tile([P, NKD, HD], FP8, name="wkb")

            def dma_x16(qc):
                cs = slice(qc * 512, (qc + 1) * 512)
                for lohi in range(2):
                    t = xtp.tile([P, 4, 512], BF16, name="xq")
                    nc.sync.dma_start(
                        out=t[:],
                        in_=xT[lohi * 512 : (lohi + 1) * 512, cs].rearrange(
                            "(k p) s -> p k s", p=P
                        ),
                    )
                    xq[qc][lohi] = t

            def dma_x8(qc):
                cs = slice(qc * 512, (qc + 1) * 512)
                t8 = x8p.tile([P, 8, 512], FP8, name="x8q")
                nc.sync.dma_start(
                    out=t8[:], in_=x8[:, cs].rearrange("(k p) s -> p k s", p=P)
                )
                x8q[qc] = t8

            nc.sync.dma_start(out=wq_big[:], in_=Wq.rearrange("(k p) c -> p k c", p=P))
            dma_x16(0)
            nc.sync.dma_start(out=wk8_big[:], in_=Wk8.rearrange("(k p) c -> p k c", p=P))
            dma_x8(0)
            nc.sync.dma_start(out=bq_sb[:], in_=bqt[:, :])
            nc.sync.dma_start(out=bk_sb[:], in_=bkt[:, :])
            for qc in range(1, NQ):
                dma_x16(qc)
                dma_x8(qc)
            for p in range(2):
                nc.sync.dma_start(
                    out=w0_sb[p][:],
                    in_=W0[p * P : (p + 1) * P, :],
                )

            # ---- sweep emitters (as drip-able item lists)
            def q_sweep_items(ni, mi):
                ps = mmp.tile([P, 512], F32, name="ps")

                def mk(k):
                    def go():
                        nc.tensor.matmul(
                            ps[:],
                            lhsT=wq_big[:, k, mi * P : (mi + 1) * P],
                            rhs=xq[ni][k // 4][:, k % 4, :],
                            start=(k == 0),
                            stop=(k == NKD - 1),
                        )

                    return go

                def evict():
                    nc.vector.tensor_scalar_add(
                        QT[mi][ni][:, :], ps[:], bq_sb[:, mi : mi + 1]
                    )
                    nc.gpsimd.tensor_copy(QT8[mi][ni][:, 0, :], QT[mi][ni][:, :])

                return [(2, mk(k)) for k in range(NKD)] + [(1, evict)]

            def k_sweep_items(ni, mi):
                ps = mmp.tile([P, 512], F32, name="ps")

                def mk(kp):
                    def go():
                        nc.tensor.matmul(
                            ps[:],
                            lhsT=wk8_big[:, 2 * kp : 2 * kp + 2, mi * P : (mi + 1) * P],
                            rhs=x8q[ni][:, 2 * kp : 2 * kp + 2, :],
                            start=(kp == 0),
                            stop=(kp == 3),
                            perf_mode=DR,
                        )

                    return go

                def evict():
                    nc.vector.tensor_scalar(
                        out=KT8[mi][ni][:, 0, :],
                        in0=ps[:],
                        scalar1=1.0 / 128.0,
                        scalar2=bk_sb[:, mi : mi + 1],
                        op0=MUL,
                        op1=ADD,
                    )

                return [(2, mk(kp)) for kp in range(4)] + [(1, evict)]

            def vT_item(pair, i):
                def go():
                    tp = tstg.tile([P, P], BF16, name="ts")
                    nc.sync.dma_start_transpose(
                        out=tp[:], in_=QT[pair][i // 4][:, (i % 4) * P : (i % 4 + 1) * P]
                    )
                    v = vt[(pair, i)]
                    nc.vector.tensor_copy(v[:, 0, 0:64], tp[:, 0:64])
                    nc.vector.tensor_copy(v[:, 1, 0:64], tp[:, 64:128])

                return (2, go)

            # ---- drip queue: background emit-thunks (sweeps, V'T, outproj)
            bg = []

            def drip(budget):
                while bg and budget > 0:
                    cost, thunk = bg.pop(0)
                    thunk()
                    budget -= cost

            def emit_outproj(m):
                state = {}

                def half(n):
                    ps = mmp.tile([P, 512], F32, name="ps")
                    for p_ in range(2):
                        nc.tensor.matmul(
                            ps[:],
                            lhsT=attnT[(p_, m)][:],
                            rhs=w0_sb[p_][:, n * 512 : (n + 1) * 512],
                            start=(p_ == 0),
                            stop=(p_ == 1),
                        )
                    nc.vector.tensor_copy(state["ot"][:, n * 512 : (n + 1) * 512], ps[:])

                def go0():
                    state["ot"] = ostp.tile([P, D], BF16, name="ot")
                    half(0)

                def go1():
                    half(1)
                    nc.sync.dma_start(out=out[m * P : (m + 1) * P, :], in_=state["ot"][:])

                return [(2, go0), (2, go1)]

            # ---- attention
            def S_mm(pair, j, i):
                """score matmuls for tile (j, i): S^T doubled via the stride-0
                second k-tile; the 2x and 1/sqrt(DK) sit in the exp scale.
                A lands at [off:512], B at [512:512+w] so one exp covers both."""
                off = max(0, i * P - j * 512)
                w = 512 - off
                kc = slice((i % 4) * P, (i % 4 + 1) * P)
                sAB = sps.tile([P, 1024], F32, name="sab")
                qs = slice(off, 512)
                for h in range(2):
                    hs = slice(h * 64, h * 64 + 64)
                    dst = sAB[:, off:512] if h == 0 else sAB[:, 512 : 512 + w]
                    nc.tensor.matmul(
                        dst,
                        lhsT=KT8[pair][i // 4][hs, :, kc].broadcast_to([64, 2, P]),
                        rhs=QT8[pair][j][hs, :, qs].broadcast_to([64, 2, w]),
                        perf_mode=DR,
                    )
                return sAB

            def S_exp(pair, j, i, sAB):
                """one exp (+ causal masks) for tile (j, i); returns probs."""
                off = max(0, i * P - j * 512)
                w = 512 - off
                pAB = ppool.tile([P, 1024], BF16, name="pab")
                nc.scalar.activation(
                    out=pAB[:, off : 512 + w],
                    in_=sAB[:, off : 512 + w],
                    func=EXP,
                    scale=0.0625,
                )
                if i >= 4 * j:  # diagonal tile: mask the leading 128-col block
                    nc.gpsimd.tensor_mul(
                        pAB[:, off : off + P], pAB[:, off : off + P], tri[:]
                    )
                    nc.gpsimd.tensor_mul(
                        pAB[:, 512 : 512 + P], pAB[:, 512 : 512 + P], tri[:]
                    )
                return pAB

            def av_mm(pair, att, s, m, j, i, pAB):
                # each att bank holds one accumulation GROUP spanning both m
                # slots: start only zeroes once (it clears the whole 2KB zero
                # region), stop only on the very last write to the bank
                cm = (m - 4 * j) * P
                off = max(0, i * P - j * 512)
                last = None
                for h in range(2):
                    lo = cm if h == 0 else 512 + cm - off
                    base = (2 * s + h) * 65
                    last = nc.tensor.matmul(
                        att[:, base : base + 65],
                        lhsT=pAB[:, lo : lo + P],
                        rhs=vt[(pair, i)][:, h, :],
                        start=(i == 0 and s == 0 and h == 0),
                        stop=(i == m and s == 1 and h == 1),
                    )
                return last

            def normalize(pair, att, s, m, dep=None):
                an = anp.tile([P, P], BF16, name="an")
                rc = rcp.tile([P, 2], F32, name="rc")
                for h in range(2):
                    base = (2 * s + h) * 65
                    r = nc.vector.reciprocal(
                        rc[:, h : h + 1], att[:, base + 64 : base + 65]
                    )
                    if dep is not None and h == 0:
                        # slot-0 values are final, but the bank's accumulation
                        # group only closes at the slot-1 stop matmul; DVE is
                        # in-order so one dep covers the whole normalize
                        add_dep_helper(r.ins, dep.ins, sync=True,
                                       reason="att group close")
                    nc.vector.tensor_scalar(
                        out=an[:, h * 64 : (h + 1) * 64],
                        in0=att[:, base : base + 64],
                        scalar1=rc[:, h : h + 1],
                        scalar2=None,
                        op0=MUL,
                    )
                nc.sync.dma_start_transpose(out=attnT[(pair, m)][:], in_=an[:])

            def av_step(j, ms, att, ip, probs, op, pair=None):
                raise NotImplementedError

            def emit_pair(pair, jorder, budget):
                def av_step(j, ms, att, ip, probs, op):
                    for m in ms:
                        if m < ip:
                            continue
                        t, s = att[m]
                        stop = av_mm(pair, t, s, m, j, ip, probs[ip])
                        if ip == m and s == 1:
                            # group closed: normalize both slots of this bank
                            normalize(pair, t, 0, m - 1, dep=stop)
                            normalize(pair, t, 1, m)
                            if pair == 1:
                                op(m - 1)
                                op(m)

                # scores run one step ahead of exps (lead-1) so the exp's
                # input semaphore has fired long before ACT gets there
                seq = [(j, i) for j in jorder for i in range(4 * j + 4)]
                sq = {}
                sq[seq[0]] = S_mm(pair, *seq[0])
                idx = 0
                for j in jorder:
                    last = pair == 1 and j == jorder[-1]

                    def op(m, last=last):
                        if last:
                            for _, it in emit_outproj(m):
                                it()
                        else:
                            bg.extend(emit_outproj(m))

                    nsteps = 4 * j + 4
                    probs = {}
                    ms = list(range(4 * j, 4 * j + 4))
                    att = {}  # m -> (tile, slot)
                    pend = []  # i's whose AV is not yet emitted
                    for i in range(nsteps):
                        if idx + 1 < len(seq):
                            sq[seq[idx + 1]] = S_mm(pair, *seq[idx + 1])
                        probs[i] = S_exp(pair, j, i, sq.pop((j, i)))
                        idx += 1
                        pend.append(i)
                        # scale the dripped background work to this step's exp
                        # length so the PE never outruns ACT on short tiles
                        w = 512 - max(0, i * P - j * 512)
                        drip(max(2, budget * (512 + w) // 1024))
                        if i == 0:
                            lo = aps.tile([P, 260], F32, name="att")
                            hi = aps.tile([P, 260], F32, name="att")
                            for s, m in enumerate(ms):
                                att[m] = (lo, s) if s < 2 else (hi, s - 2)
                        if i >= 1:
                            ip = pend.pop(0)
                            av_step(j, ms, att, ip, probs, op)
                    while pend:
                        ip = pend.pop(0)
                        av_step(j, ms, att, ip, probs, op)

            # ---- schedule
            # upfront: first Q/K sweeps + first V' transposes (gate the first
            # score tile), everything else drips
            for _, it in q_sweep_items(0, 0):
                it()
            for _, it in k_sweep_items(0, 0):
                it()
            for i in range(4):
                _, it = vT_item(0, i)
                it()

            order = []
            for ni in (1, 2, 3):
                order += q_sweep_items(ni, 0) + k_sweep_items(ni, 0)
                order += [vT_item(0, i) for i in range(4 * ni, 4 * ni + 4)]
            for ni in range(4):
                order += q_sweep_items(ni, 1) + k_sweep_items(ni, 1)
                order += [vT_item(1, i) for i in range(4 * ni, 4 * ni + 4)]
            bg.extend(order)

            emit_pair(0, (0, 1, 2, 3), budget=7)
            emit_pair(1, (0, 1, 2, 3), budget=5)
            while bg:
                drip(6)

    nc.compile()
    return nc


def make_in_maps(pos_encode_toks, Wq, bq, Wk, bk, W0, b0):
    x = np.asarray(pos_encode_toks, dtype=np.float32)
    Wq = np.asarray(Wq, dtype=np.float32)
    bq = np.asarray(bq, dtype=np.float32)
    Wk = np.asarray(Wk, dtype=np.float32)
    bk = np.asarray(bk, dtype=np.float32)
    W0 = np.asarray(W0, dtype=np.float32)
    in_maps = []
    for core in range(8):
        b, g = divmod(core, 4)
        hs = slice(g * HD, (g + 1) * HD)
        xt = np.ascontiguousarray(x[b].T)
        in_maps.append(
            {
                "xT": xt.astype(ml_dtypes.bfloat16),
                "x8": xt.astype(ml_dtypes.float8_e4m3),
                "Wq": np.ascontiguousarray(Wq[:, hs]).astype(ml_dtypes.bfloat16),
                "Wk8": np.ascontiguousarray(Wk[:, hs] * 128.0).astype(
                    ml_dtypes.float8_e4m3
                ),
                "bqt": np.ascontiguousarray(bq[hs].reshape(2, P).T),
                "bkt": np.ascontiguousarray(bk[hs].reshape(2, P).T),
                "W0": np.ascontiguousarray(W0[hs, :]).astype(ml_dtypes.bfloat16),
            }
        )
    return in_maps


def assemble(results, b0):
    out = np.zeros((2, S, D), dtype=np.float32)
    for core in range(8):
        b = core // 4
        out[b] += results[core]["out"].astype(np.float32)
    out += np.asarray(b0, dtype=np.float32)
    return out


def kernel(pos_encode_toks, Wq, bq, Wk, bk, W0, b0):
    from concourse.bass_utils import run_bass_kernel_spmd

    global _CACHED_NC
    if _CACHED_NC is None:
        _CACHED_NC = build_nc()
    in_maps = make_in_maps(pos_encode_toks, Wq, bq, Wk, bk, W0, b0)
    res = run_bass_kernel_spmd(_CACHED_NC, in_maps, core_ids=list(range(8)))
    return assemble(res.results, b0)


# revision 5
# speedup vs baseline: 1.3070x; 1.0581x over previous
"""Multi-head causal self-attention (V=Q variant) on 8 Trainium2 cores, v2.

Sharding: batch (2) x head-group (4 groups of 4 heads). Each core computes
full-sequence attention for its 4 heads (2 head-pairs) of one batch element
plus its slice of the output projection; the host sums 4 partials per batch
and adds b0.

v2 redesign vs v1:
- K projection and scores run in fp8e4m3 DoubleRow (0.5 cycles/row).
  Scores use a zeroed second k-tile so a 64-deep contraction still gets the
  DoubleRow rate. Wk is host-prescaled by 128 to keep fp8 normals; the
  1/128 and the 1/sqrt(DK) are folded into the K eviction and exp scale.
- AV is computed reversed: out[q,65] = probs_chunk^T @ [V|1], filling all
  128 output partitions per pass (the forward form only filled 65). The
  softmax denominator still falls out of the ones column; normalization is
  a per-partition divide (Pool), then attn^T is rebuilt with one PE
  transpose per (pair, m) for the output projection.
- exp is merged to one [128,1024] activation per full score tile (A and B
  halves share a 2-bank PSUM tile); ACT is the bottleneck engine.
- Evictions are spread: DVE does projections/V'/attnT, Pool does normalize
  divides and output-projection evictions.
"""

import ml_dtypes
import numpy as np

import concourse.bacc as bacc
import concourse.mybir as mybir
from concourse.tile import TileContext, add_dep_helper

P = 128
S = 2048
D = 1024
HD = 256
DK = 64
NQ = 4  # q blocks of 512
NKD = 8  # D chunks of 128
F32 = mybir.dt.float32
BF16 = mybir.dt.bfloat16
FP8 = mybir.dt.float8e4
EXP = mybir.ActivationFunctionType.Exp
DR = mybir.MatmulPerfMode.DoubleRow
MUL = mybir.AluOpType.mult
ADD = mybir.AluOpType.add
DIV = mybir.AluOpType.divide

_CACHED_NC = None


def build_nc():
    nc = bacc.Bacc("TRN2", target_bir_lowering=False, debug=False, num_devices=8)
    xT = nc.declare_dram_parameter("xT", [D, S], BF16, isOutput=False)
    x8 = nc.declare_dram_parameter("x8", [D, S], FP8, isOutput=False)
    Wq = nc.declare_dram_parameter("Wq", [D, HD], BF16, isOutput=False)
    Wk8 = nc.declare_dram_parameter("Wk8", [D, HD], FP8, isOutput=False)
    bqt = nc.declare_dram_parameter("bqt", [P, 2], F32, isOutput=False)
    bkt = nc.declare_dram_parameter("bkt", [P, 2], F32, isOutput=False)
    W0 = nc.declare_dram_parameter("W0", [HD, D], BF16, isOutput=False)
    out = nc.declare_dram_parameter("out", [S, D], BF16, isOutput=True)

    with TileContext(nc) as tc:
        with (
            tc.tile_pool(name="const", bufs=1) as const,
            tc.tile_pool(name="xt", bufs=8) as xtp,
            tc.tile_pool(name="x8p", bufs=4) as x8p,
            tc.tile_pool(name="pp", bufs=24) as ppool,
            tc.tile_pool(name="an", bufs=4) as anp,
            tc.tile_pool(name="tst", bufs=3) as tstg,
            tc.tile_pool(name="rc", bufs=4) as rcp,
            tc.tile_pool(name="ost", bufs=3) as ostp,
            tc.tile_pool(name="sps", bufs=2, space="PSUM") as sps,
            tc.tile_pool(name="aps", bufs=2, space="PSUM") as aps,
            tc.tile_pool(name="mm", bufs=2, space="PSUM") as mmp,
        ):
            # ---- PE p-state warmup first: dummy matmuls keep the PE busy
            # through the initial DMA wait so real matmuls start at full clock
            wa = const.tile([P, 256], BF16, name="wa")
            nc.gpsimd.memset(wa[:], 0.0)
            for _ in range(14):
                wps = mmp.tile([P, 256], F32, name="ps")
                nc.tensor.matmul(wps[:], lhsT=wa[:, 0:128], rhs=wa[:], start=True, stop=True)

            # ---- constants / static tiles
            identity = const.tile([P, P], BF16)
            nc.gpsimd.memset(identity[:], 0.0)
            nc.gpsimd.affine_select(
                out=identity[:],
                in_=identity[:],
                compare_op=mybir.AluOpType.not_equal,
                fill=1.0,
                base=0,
                pattern=[[-1, P]],
                channel_multiplier=1,
            )
            # triangular keep-mask [128,128]: 1.0 where q >= kv
            tri = const.tile([P, P], BF16, name="tri")
            nc.gpsimd.memset(tri[:], 1.0)
            nc.gpsimd.affine_select(
                out=tri[:],
                in_=tri[:],
                compare_op=mybir.AluOpType.is_ge,
                fill=0.0,
                base=0,
                pattern=[[1, P]],
                channel_multiplier=-1,
            )
            # ACT exp-table warmup while DMAs run
            warm = const.tile([P, 8], F32, name="warm")
            nc.gpsimd.memset(warm[:], 0.0)
            nc.scalar.activation(out=warm[:], in_=warm[:], func=EXP)

            bq_sb = const.tile([P, 2], F32)
            bk_sb = const.tile([P, 2], F32)
            w0_sb = [const.tile([P, D], BF16, name=f"w0_{p}") for p in range(2)]
            # projections: QT (bf16, feeds V' transposes), QT8/KT8 (fp8;
            # scores use a stride-0 broadcast second k-tile, doubling the
            # result -- absorbed by the exp scale)
            QT = [
                [const.tile([P, 512], BF16, name=f"qt{mi}_{ni}") for ni in range(NQ)]
                for mi in range(2)
            ]
            QT8 = [
                [const.tile([P, 1, 512], FP8, name=f"qt8{mi}_{ni}") for ni in range(NQ)]
                for mi in range(2)
            ]
            KT8 = [
                [const.tile([P, 1, 512], FP8, name=f"kt8{mi}_{ni}") for ni in range(NQ)]
                for mi in range(2)
            ]
            vt = {}
            for pair in range(2):
                for i in range(16):
                    vt[(pair, i)] = const.tile([P, 2, 65], BF16, name=f"vt{pair}_{i}")
            attnT = {}  # (pair, m-pair) -> [P, 2, P]; [:, m%2, :] = attn^T(m)
            for pair in range(2):
                for mp in range(8):
                    attnT[(pair, mp)] = const.tile([P, 2, P], BF16, name=f"at{pair}_{mp}")

            # ones column of every V' tile (Pool, pair-0 first); written once
            for key in vt:
                nc.gpsimd.memset(vt[key][:, :, 64:65], 1.0)

            # ---- DMAs, ordered so the first Q/K sweeps can start early.
            # x arrives in 512-column (q-chunk) slabs, k-major inside.
            xq = [[None, None] for _ in range(NQ)]  # [qc][lohi] -> [P, 4, 512]
            x8q = [None] * NQ  # [qc] -> [P, 8, 512] fp8
            wq_big = const.tile([P, NKD, HD], BF16, name="wqb")
            wk8_big = const.tile([P, NKD, HD], FP8, name="wkb")

            def dma_x16(qc):
                cs = slice(qc * 512, (qc + 1) * 512)
                for lohi in range(2):
                    t = xtp.tile([P, 4, 512], BF16, name="xq")
                    nc.sync.dma_start(
                        out=t[:],
                        in_=xT[lohi * 512 : (lohi + 1) * 512, cs].rearrange(
                            "(k p) s -> p k s", p=P
                        ),
                    )
                    xq[qc][lohi] = t

            def dma_x8(qc):
                cs = slice(qc * 512, (qc + 1) * 512)
                t8 = x8p.tile([P, 8, 512], FP8, name="x8q")
                nc.sync.dma_start(
                    out=t8[:], in_=x8[:, cs].rearrange("(k p) s -> p k s", p=P)
                )
                x8q[qc] = t8

            nc.sync.dma_start(out=wq_big[:], in_=Wq.rearrange("(k p) c -> p k c", p=P))
            dma_x16(0)
            nc.sync.dma_start(out=wk8_big[:], in_=Wk8.rearrange("(k p) c -> p k c", p=P))
            dma_x8(0)
            nc.sync.dma_start(out=bq_sb[:], in_=bqt[:, :])
            nc.sync.dma_start(out=bk_sb[:], in_=bkt[:, :])
            for qc in range(1, NQ):
                dma_x16(qc)
                dma_x8(qc)
            for p in range(2):
                nc.sync.dma_start(
                    out=w0_sb[p][:],
                    in_=W0[p * P : (p + 1) * P, :],
                )

            # ---- sweep emitters (as drip-able item lists)
            def q_sweep_items(ni, mi):
                ps = mmp.tile([P, 512], F32, name="ps")

                def mk(k):
                    def go():
                        nc.tensor.matmul(
                            ps[:],
                            lhsT=wq_big[:, k, mi * P : (mi + 1) * P],
                            rhs=xq[ni][k // 4][:, k % 4, :],
                            start=(k == 0),
                            stop=(k == NKD - 1),
                        )

                    return go

                def evict():
                    nc.vector.tensor_scalar_add(
                        QT[mi][ni][:, :], ps[:], bq_sb[:, mi : mi + 1]
                    )
                    nc.gpsimd.tensor_copy(QT8[mi][ni][:, 0, :], QT[mi][ni][:, :])

                return [(2, mk(k)) for k in range(NKD)] + [(1, evict)]

            def k_sweep_items(ni, mi):
                ps = mmp.tile([P, 512], F32, name="ps")

                def mk(kp):
                    def go():
                        nc.tensor.matmul(
                            ps[:],
                            lhsT=wk8_big[:, 2 * kp : 2 * kp + 2, mi * P : (mi + 1) * P],
                            rhs=x8q[ni][:, 2 * kp : 2 * kp + 2, :],
                            start=(kp == 0),
                            stop=(kp == 3),
                            perf_mode=DR,
                        )

                    return go

                def evict():
                    nc.vector.tensor_scalar(
                        out=KT8[mi][ni][:, 0, :],
                        in0=ps[:],
                        scalar1=1.0 / 128.0,
                        scalar2=bk_sb[:, mi : mi + 1],
                        op0=MUL,
                        op1=ADD,
                    )

                return [(2, mk(kp)) for kp in range(4)] + [(1, evict)]

            def vT_items(pair, ni):
                """one batched DMA transpose per QT tile covers 4 V' chunks"""
                state = {}

                def tp_go():
                    state["tp"] = tstg.tile([P, 4, P], BF16, name="ts")
                    nc.sync.dma_start_transpose(out=state["tp"][:], in_=QT[pair][ni][:, :])

                def cp(c):
                    def go():
                        v = vt[(pair, 4 * ni + c)]
                        src_ap = state["tp"][:, c, :].rearrange("p (h d) -> p h d", h=2)
                        nc.vector.tensor_copy(v[:, :, 0:64], src_ap)

                    return go

                return [(2, tp_go)] + [(1, cp(c)) for c in range(4)]

            # ---- drip queue: background emit-thunks (sweeps, V'T, outproj)
            bg = []

            def drip(budget):
                while bg and budget > 0:
                    cost, thunk = bg.pop(0)
                    thunk()
                    budget -= cost

            def emit_outproj(m):
                state = {}

                def half(n):
                    ps = mmp.tile([P, 512], F32, name="ps")
                    for p_ in range(2):
                        nc.tensor.matmul(
                            ps[:],
                            lhsT=attnT[(p_, m // 2)][:, m % 2, :],
                            rhs=w0_sb[p_][:, n * 512 : (n + 1) * 512],
                            start=(p_ == 0),
                            stop=(p_ == 1),
                        )
                    nc.vector.tensor_copy(state["ot"][:, n * 512 : (n + 1) * 512], ps[:])

                def go0():
                    state["ot"] = ostp.tile([P, D], BF16, name="ot")
                    half(0)

                def go1():
                    half(1)
                    nc.sync.dma_start(out=out[m * P : (m + 1) * P, :], in_=state["ot"][:])

                return [(2, go0), (2, go1)]

            # ---- attention
            def S_mm(pair, j, i):
                """score matmuls for tile (j, i): S^T doubled via the stride-0
                second k-tile; the 2x and 1/sqrt(DK) sit in the exp scale.
                A lands at [off:512], B at [512:512+w] so one exp covers both."""
                off = max(0, i * P - j * 512)
                w = 512 - off
                kc = slice((i % 4) * P, (i % 4 + 1) * P)
                sAB = sps.tile([P, 1024], F32, name="sab")
                qs = slice(off, 512)
                for h in range(2):
                    hs = slice(h * 64, h * 64 + 64)
                    dst = sAB[:, off:512] if h == 0 else sAB[:, 512 : 512 + w]
                    nc.tensor.matmul(
                        dst,
                        lhsT=KT8[pair][i // 4][hs, :, kc].broadcast_to([64, 2, P]),
                        rhs=QT8[pair][j][hs, :, qs].broadcast_to([64, 2, w]),
                        perf_mode=DR,
                    )
                return sAB

            def S_exp(pair, j, i, sAB):
                """one exp (+ causal masks) for tile (j, i); returns probs."""
                off = max(0, i * P - j * 512)
                w = 512 - off
                pAB = ppool.tile([P, 1024], BF16, name="pab")
                nc.scalar.activation(
                    out=pAB[:, off : 512 + w],
                    in_=sAB[:, off : 512 + w],
                    func=EXP,
                    scale=0.0625,
                )
                if i >= 4 * j:  # diagonal tile: mask the leading 128-col block
                    nc.gpsimd.tensor_mul(
                        pAB[:, off : off + P], pAB[:, off : off + P], tri[:]
                    )
                    nc.gpsimd.tensor_mul(
                        pAB[:, 512 : 512 + P], pAB[:, 512 : 512 + P], tri[:]
                    )
                return pAB

            def av_mm(pair, att, s, m, j, i, pAB):
                # each att bank holds one accumulation GROUP spanning both m
                # slots: start only zeroes once (it clears the whole 2KB zero
                # region), stop only on the very last write to the bank
                cm = (m - 4 * j) * P
                off = max(0, i * P - j * 512)
                last = None
                for h in range(2):
                    lo = cm if h == 0 else 512 + cm - off
                    base = (2 * s + h) * 65
                    last = nc.tensor.matmul(
                        att[:, base : base + 65],
                        lhsT=pAB[:, lo : lo + P],
                        rhs=vt[(pair, i)][:, h, :],
                        start=(i == 0 and s == 0 and h == 0),
                        stop=(i == m and s == 1 and h == 1),
                    )
                return last

            def normalize(pair, att, s, m, an, dep=None):
                rc = rcp.tile([P, 2], F32, name="rc")
                for h in range(2):
                    base = (2 * s + h) * 65
                    r = nc.vector.reciprocal(
                        rc[:, h : h + 1], att[:, base + 64 : base + 65]
                    )
                    if dep is not None and h == 0:
                        # slot-0 values are final, but the bank's accumulation
                        # group only closes at the slot-1 stop matmul; DVE is
                        # in-order so one dep covers the whole normalize
                        add_dep_helper(r.ins, dep.ins, sync=True,
                                       reason="att group close")
                    nc.vector.tensor_scalar(
                        out=an[:, 128 * s + h * 64 : 128 * s + (h + 1) * 64],
                        in0=att[:, base : base + 64],
                        scalar1=rc[:, h : h + 1],
                        scalar2=None,
                        op0=MUL,
                    )

            def av_step(j, ms, att, ip, probs, op, pair=None):
                raise NotImplementedError

            def emit_pair(pair, jorder, budget):
                def av_step(j, ms, att, ip, probs, op):
                    for m in ms:
                        if m < ip:
                            continue
                        t, s = att[m]
                        stop = av_mm(pair, t, s, m, j, ip, probs[ip])
                        if ip == m and s == 1:
                            # group closed: normalize both slots of this bank,
                            # then one batched transpose covers the m-pair
                            an = anp.tile([P, 256], BF16, name="an")
                            normalize(pair, t, 0, m - 1, an, dep=stop)
                            normalize(pair, t, 1, m, an)
                            nc.sync.dma_start_transpose(
                                out=attnT[(pair, m // 2)][:], in_=an[:]
                            )
                            if pair == 1:
                                op(m - 1)
                                op(m)

                # scores run one step ahead of exps (lead-1) so the exp's
                # input semaphore has fired long before ACT gets there
                seq = [(j, i) for j in jorder for i in range(4 * j + 4)]
                sq = {}
                sq[seq[0]] = S_mm(pair, *seq[0])
                idx = 0
                for j in jorder:
                    last = pair == 1 and j == jorder[-1]

                    def op(m, last=last):
                        if last:
                            for _, it in emit_outproj(m):
                                it()
                        else:
                            bg.extend(emit_outproj(m))

                    nsteps = 4 * j + 4
                    probs = {}
                    ms = list(range(4 * j, 4 * j + 4))
                    att = {}  # m -> (tile, slot)
                    pend = []  # i's whose AV is not yet emitted
                    for i in range(nsteps):
                        if idx + 1 < len(seq):
                            sq[seq[idx + 1]] = S_mm(pair, *seq[idx + 1])
                        probs[i] = S_exp(pair, j, i, sq.pop((j, i)))
                        idx += 1
                        pend.append(i)
                        # scale the dripped background work to this step's exp
                        # length so the PE never outruns ACT on short tiles
                        w = 512 - max(0, i * P - j * 512)
                        drip(max(2, budget * (512 + w) // 1024))
                        if i == 0:
                            lo = aps.tile([P, 260], F32, name="att")
                            hi = aps.tile([P, 260], F32, name="att")
                            for s, m in enumerate(ms):
                                att[m] = (lo, s) if s < 2 else (hi, s - 2)
                        if i >= 1:
                            ip = pend.pop(0)
                            av_step(j, ms, att, ip, probs, op)
                    while pend:
                        ip = pend.pop(0)
                        av_step(j, ms, att, ip, probs, op)

            # ---- schedule
            # upfront: first Q/K sweeps + first V' transposes (gate the first
            # score tile), everything else drips
            for _, it in q_sweep_items(0, 0):
                it()
            for _, it in k_sweep_items(0, 0):
                it()
            for _, it in vT_items(0, 0):
                it()

            order = []
            for ni in (1, 2, 3):
                order += q_sweep_items(ni, 0) + k_sweep_items(ni, 0)
                order += vT_items(0, ni)
            for ni in range(4):
                order += q_sweep_items(ni, 1) + k_sweep_items(ni, 1)
                order += vT_items(1, ni)
            bg.extend(order)

            emit_pair(0, (0, 1, 2, 3), budget=7)
            emit_pair(1, (0, 1, 2, 3), budget=5)
            while bg:
                drip(6)

    nc.compile()
    return nc


def make_in_maps(pos_encode_toks, Wq, bq, Wk, bk, W0, b0):
    x = np.asarray(pos_encode_toks, dtype=np.float32)
    Wq = np.asarray(Wq, dtype=np.float32)
    bq = np.asarray(bq, dtype=np.float32)
    Wk = np.asarray(Wk, dtype=np.float32)
    bk = np.asarray(bk, dtype=np.float32)
    W0 = np.asarray(W0, dtype=np.float32)
    in_maps = []
    for core in range(8):
        b, g = divmod(core, 4)
        hs = slice(g * HD, (g + 1) * HD)
        xt = np.ascontiguousarray(x[b].T)
        in_maps.append(
            {
                "xT": xt.astype(ml_dtypes.bfloat16),
                "x8": xt.astype(ml_dtypes.float8_e4m3),
                "Wq": np.ascontiguousarray(Wq[:, hs]).astype(ml_dtypes.bfloat16),
                "Wk8": np.ascontiguousarray(Wk[:, hs] * 128.0).astype(
                    ml_dtypes.float8_e4m3
                ),
                "bqt": np.ascontiguousarray(bq[hs].reshape(2, P).T),
                "bkt": np.ascontiguousarray(bk[hs].reshape(2, P).T),
                "W0": np.ascontiguousarray(W0[hs, :]).astype(ml_dtypes.bfloat16),
            }
        )
    return in_maps


def assemble(results, b0):
    out = np.zeros((2, S, D), dtype=np.float32)
    for core in range(8):
        b = core // 4
        out[b] += results[core]["out"].astype(np.float32)
    out += np.asarray(b0, dtype=np.float32)
    return out


def kernel(pos_encode_toks, Wq, bq, Wk, bk, W0, b0):
    from concourse.bass_utils import run_bass_kernel_spmd

    global _CACHED_NC
    if _CACHED_NC is None:
        _CACHED_NC = build_nc()
    in_maps = make_in_maps(pos_encode_toks, Wq, bq, Wk, bk, W0, b0)
    res = run_bass_kernel_spmd(_CACHED_NC, in_maps, core_ids=list(range(8)))
    return assemble(res.results, b0)


# revision 6
# speedup vs baseline: 1.3313x; 1.0186x over previous
"""Multi-head causal self-attention (V=Q variant) on 8 Trainium2 cores, v2.

Sharding: batch (2) x head-group (4 groups of 4 heads). Each core computes
full-sequence attention for its 4 heads (2 head-pairs) of one batch element
plus its slice of the output projection; the host sums 4 partials per batch
and adds b0.

v2 redesign vs v1:
- K projection and scores run in fp8e4m3 DoubleRow (0.5 cycles/row).
  Scores use a zeroed second k-tile so a 64-deep contraction still gets the
  DoubleRow rate. Wk is host-prescaled by 128 to keep fp8 normals; the
  1/128 and the 1/sqrt(DK) are folded into the K eviction and exp scale.
- AV is computed reversed: out[q,65] = probs_chunk^T @ [V|1], filling all
  128 output partitions per pass (the forward form only filled 65). The
  softmax denominator still falls out of the ones column; normalization is
  a per-partition divide (Pool), then attn^T is rebuilt with one PE
  transpose per (pair, m) for the output projection.
- exp is merged to one [128,1024] activation per full score tile (A and B
  halves share a 2-bank PSUM tile); ACT is the bottleneck engine.
- Evictions are spread: DVE does projections/V'/attnT, Pool does normalize
  divides and output-projection evictions.
"""

import ml_dtypes
import numpy as np

import concourse.bacc as bacc
import concourse.mybir as mybir
from concourse.tile import TileContext, add_dep_helper

P = 128
S = 2048
D = 1024
HD = 256
DK = 64
NQ = 4  # q blocks of 512
NKD = 8  # D chunks of 128
F32 = mybir.dt.float32
BF16 = mybir.dt.bfloat16
FP8 = mybir.dt.float8e4
EXP = mybir.ActivationFunctionType.Exp
DR = mybir.MatmulPerfMode.DoubleRow
MUL = mybir.AluOpType.mult
ADD = mybir.AluOpType.add
DIV = mybir.AluOpType.divide

_CACHED_NC = None


def build_nc():
    nc = bacc.Bacc("TRN2", target_bir_lowering=False, debug=False, num_devices=8)
    xT = nc.declare_dram_parameter("xT", [D, S], BF16, isOutput=False)
    x8 = nc.declare_dram_parameter("x8", [D, S], FP8, isOutput=False)
    Wq = nc.declare_dram_parameter("Wq", [P, NKD, HD], BF16, isOutput=False)
    Wk8 = nc.declare_dram_parameter("Wk8", [P, NKD, HD], FP8, isOutput=False)
    bqt = nc.declare_dram_parameter("bqt", [P, 2], F32, isOutput=False)
    bkt = nc.declare_dram_parameter("bkt", [P, 2], F32, isOutput=False)
    W0 = nc.declare_dram_parameter("W0", [HD, D], BF16, isOutput=False)
    out = nc.declare_dram_parameter("out", [S, D], BF16, isOutput=True)

    with TileContext(nc) as tc:
        with (
            tc.tile_pool(name="const", bufs=1) as const,
            tc.tile_pool(name="xt", bufs=8) as xtp,
            tc.tile_pool(name="x8p", bufs=4) as x8p,
            tc.tile_pool(name="pp", bufs=24) as ppool,
            tc.tile_pool(name="an", bufs=4) as anp,
            tc.tile_pool(name="tst", bufs=3) as tstg,
            tc.tile_pool(name="rc", bufs=4) as rcp,
            tc.tile_pool(name="ost", bufs=3) as ostp,
            tc.tile_pool(name="sps", bufs=2, space="PSUM") as sps,
            tc.tile_pool(name="aps", bufs=2, space="PSUM") as aps,
            tc.tile_pool(name="mm", bufs=2, space="PSUM") as mmp,
        ):
            # ---- PE p-state warmup first: dummy matmuls keep the PE busy
            # through the initial DMA wait so real matmuls start at full clock
            wa = const.tile([P, 256], BF16, name="wa")
            nc.gpsimd.memset(wa[:], 0.0)
            for _ in range(14):
                wps = mmp.tile([P, 256], F32, name="ps")
                nc.tensor.matmul(wps[:], lhsT=wa[:, 0:128], rhs=wa[:], start=True, stop=True)

            # ---- constants / static tiles
            identity = const.tile([P, P], BF16)
            nc.gpsimd.memset(identity[:], 0.0)
            nc.gpsimd.affine_select(
                out=identity[:],
                in_=identity[:],
                compare_op=mybir.AluOpType.not_equal,
                fill=1.0,
                base=0,
                pattern=[[-1, P]],
                channel_multiplier=1,
            )
            # triangular keep-mask [128,128]: 1.0 where q >= kv
            tri = const.tile([P, P], BF16, name="tri")
            nc.gpsimd.memset(tri[:], 1.0)
            nc.gpsimd.affine_select(
                out=tri[:],
                in_=tri[:],
                compare_op=mybir.AluOpType.is_ge,
                fill=0.0,
                base=0,
                pattern=[[1, P]],
                channel_multiplier=-1,
            )
            # ACT exp-table warmup while DMAs run
            warm = const.tile([P, 8], F32, name="warm")
            nc.gpsimd.memset(warm[:], 0.0)
            nc.scalar.activation(out=warm[:], in_=warm[:], func=EXP)

            bq_sb = const.tile([P, 2], F32)
            bk_sb = const.tile([P, 2], F32)
            w0_sb = [const.tile([P, D], BF16, name=f"w0_{p}") for p in range(2)]
            # projections: QT (bf16, feeds V' transposes), QT8/KT8 (fp8;
            # scores use a stride-0 broadcast second k-tile, doubling the
            # result -- absorbed by the exp scale)
            QT = [
                [const.tile([P, 512], BF16, name=f"qt{mi}_{ni}") for ni in range(NQ)]
                for mi in range(2)
            ]
            QT8 = [
                [const.tile([P, 1, 512], FP8, name=f"qt8{mi}_{ni}") for ni in range(NQ)]
                for mi in range(2)
            ]
            KT8 = [
                [const.tile([P, 1, 512], FP8, name=f"kt8{mi}_{ni}") for ni in range(NQ)]
                for mi in range(2)
            ]
            vt = {}
            for pair in range(2):
                for i in range(16):
                    vt[(pair, i)] = const.tile([P, 2, 65], BF16, name=f"vt{pair}_{i}")
            attnT = {}  # (pair, m-pair) -> [P, 2, P]; [:, m%2, :] = attn^T(m)
            for pair in range(2):
                for mp in range(8):
                    attnT[(pair, mp)] = const.tile([P, 2, P], BF16, name=f"at{pair}_{mp}")

            # ones column of every V' tile (Pool, pair-0 first); written once
            for key in vt:
                nc.gpsimd.memset(vt[key][:, :, 64:65], 1.0)

            # ---- DMAs, ordered so the first Q/K sweeps can start early.
            # x arrives in 512-column (q-chunk) slabs, k-major inside.
            xq = [[None, None] for _ in range(NQ)]  # [qc][lohi] -> [P, 4, 512]
            x8q = [None] * NQ  # [qc] -> [P, 8, 512] fp8
            wq_big = const.tile([P, NKD, HD], BF16, name="wqb")
            wk8_big = const.tile([P, NKD, HD], FP8, name="wkb")

            def dma_x16(qc):
                cs = slice(qc * 512, (qc + 1) * 512)
                for lohi in range(2):
                    t = xtp.tile([P, 4, 512], BF16, name="xq")
                    nc.sync.dma_start(
                        out=t[:],
                        in_=xT[lohi * 512 : (lohi + 1) * 512, cs].rearrange(
                            "(k p) s -> p k s", p=P
                        ),
                    )
                    xq[qc][lohi] = t

            def dma_x8(qc):
                cs = slice(qc * 512, (qc + 1) * 512)
                t8 = x8p.tile([P, 8, 512], FP8, name="x8q")
                nc.sync.dma_start(
                    out=t8[:], in_=x8[:, cs].rearrange("(k p) s -> p k s", p=P)
                )
                x8q[qc] = t8

            nc.sync.dma_start(out=wq_big[:], in_=Wq[:, :, :])
            dma_x16(0)
            nc.sync.dma_start(out=bq_sb[:], in_=bqt[:, :])
            nc.sync.dma_start(out=bk_sb[:], in_=bkt[:, :])
            nc.sync.dma_start(out=wk8_big[:], in_=Wk8[:, :, :])
            dma_x8(0)
            for qc in range(1, NQ):
                dma_x16(qc)
                dma_x8(qc)
            for p in range(2):
                nc.sync.dma_start(
                    out=w0_sb[p][:],
                    in_=W0[p * P : (p + 1) * P, :],
                )

            # ---- sweep emitters (as drip-able item lists)
            def q_sweep_items(ni, mi):
                ps = mmp.tile([P, 512], F32, name="ps")

                def mk(k):
                    def go():
                        nc.tensor.matmul(
                            ps[:],
                            lhsT=wq_big[:, k, mi * P : (mi + 1) * P],
                            rhs=xq[ni][k // 4][:, k % 4, :],
                            start=(k == 0),
                            stop=(k == NKD - 1),
                        )

                    return go

                def evict():
                    # two direct evictions: fp8 for scores (critical path),
                    # bf16 for the V' transposes
                    nc.vector.tensor_scalar_add(
                        QT8[mi][ni][:, 0, :], ps[:], bq_sb[:, mi : mi + 1]
                    )
                    nc.vector.tensor_scalar_add(
                        QT[mi][ni][:, :], ps[:], bq_sb[:, mi : mi + 1]
                    )

                return [(2, mk(k)) for k in range(NKD)] + [(1, evict)]

            def k_sweep_items(ni, mi):
                ps = mmp.tile([P, 512], F32, name="ps")

                def mk(kp):
                    def go():
                        nc.tensor.matmul(
                            ps[:],
                            lhsT=wk8_big[:, 2 * kp : 2 * kp + 2, mi * P : (mi + 1) * P],
                            rhs=x8q[ni][:, 2 * kp : 2 * kp + 2, :],
                            start=(kp == 0),
                            stop=(kp == 3),
                            perf_mode=DR,
                        )

                    return go

                def evict():
                    nc.vector.tensor_scalar(
                        out=KT8[mi][ni][:, 0, :],
                        in0=ps[:],
                        scalar1=1.0 / 128.0,
                        scalar2=bk_sb[:, mi : mi + 1],
                        op0=MUL,
                        op1=ADD,
                    )

                return [(2, mk(kp)) for kp in range(4)] + [(1, evict)]

            def vT_items(pair, ni):
                """one batched DMA transpose per QT tile covers 4 V' chunks"""
                state = {}

                def tp_go():
                    state["tp"] = tstg.tile([P, 4, P], BF16, name="ts")
                    nc.sync.dma_start_transpose(out=state["tp"][:], in_=QT[pair][ni][:, :])

                def cp(c):
                    def go():
                        v = vt[(pair, 4 * ni + c)]
                        src_ap = state["tp"][:, c, :].rearrange("p (h d) -> p h d", h=2)
                        nc.vector.tensor_copy(v[:, :, 0:64], src_ap)

                    return go

                return [(2, tp_go)] + [(1, cp(c)) for c in range(4)]

            # ---- drip queue: background emit-thunks (sweeps, V'T, outproj)
            # items may carry a min step number (global exp-step counter) so
            # work that waits on a fresh DMA-transpose isn't popped while its
            # input is still in flight (it would stall the in-order PE queue)
            bg = []
            stepno = [0]

            def drip(budget):
                i2 = 0
                while i2 < len(bg) and budget > 0:
                    item = bg[i2]
                    if len(item) == 3 and item[2] > stepno[0]:
                        i2 += 1
                        continue
                    bg.pop(i2)
                    item[1]()
                    budget -= item[0]

            def emit_outproj(m, endgame=False):
                # endgame (post-last-exp): evictions alternate ACT/DVE (ACT is
                # idle by then) and the out DMA goes per-half to start earlier
                state = {}

                def half(n):
                    ps = mmp.tile([P, 512], F32, name="ps")
                    for p_ in range(2):
                        nc.tensor.matmul(
                            ps[:],
                            lhsT=attnT[(p_, m // 2)][:, m % 2, :],
                            rhs=w0_sb[p_][:, n * 512 : (n + 1) * 512],
                            start=(p_ == 0),
                            stop=(p_ == 1),
                        )
                    dst = state["ot"][:, n * 512 : (n + 1) * 512]
                    if endgame and n == 0:
                        nc.scalar.copy(dst, ps[:])
                    else:
                        nc.vector.tensor_copy(dst, ps[:])

                def go0():
                    state["ot"] = ostp.tile([P, D], BF16, name="ot")
                    half(0)
                    if endgame:
                        nc.sync.dma_start(
                            out=out[m * P : (m + 1) * P, 0:512],
                            in_=state["ot"][:, 0:512],
                        )

                def go1():
                    half(1)
                    if endgame:
                        nc.sync.dma_start(
                            out=out[m * P : (m + 1) * P, 512:1024],
                            in_=state["ot"][:, 512:1024],
                        )
                    else:
                        nc.sync.dma_start(
                            out=out[m * P : (m + 1) * P, :], in_=state["ot"][:]
                        )

                return [(2, go0), (2, go1)]

            # ---- attention
            def S_mm(pair, j, i):
                """score matmuls for tile (j, i): S^T doubled via the stride-0
                second k-tile; the 2x and 1/sqrt(DK) sit in the exp scale.
                A lands at [off:512], B at [512:512+w] so one exp covers both."""
                off = max(0, i * P - j * 512)
                w = 512 - off
                kc = slice((i % 4) * P, (i % 4 + 1) * P)
                sAB = sps.tile([P, 1024], F32, name="sab")
                qs = slice(off, 512)
                for h in range(2):
                    hs = slice(h * 64, h * 64 + 64)
                    dst = sAB[:, off:512] if h == 0 else sAB[:, 512 : 512 + w]
                    nc.tensor.matmul(
                        dst,
                        lhsT=KT8[pair][i // 4][hs, :, kc].broadcast_to([64, 2, P]),
                        rhs=QT8[pair][j][hs, :, qs].broadcast_to([64, 2, w]),
                        perf_mode=DR,
                    )
                return sAB

            def S_exp(pair, j, i, sAB):
                """one exp (+ causal masks) for tile (j, i); returns probs."""
                off = max(0, i * P - j * 512)
                w = 512 - off
                pAB = ppool.tile([P, 1024], BF16, name="pab")
                nc.scalar.activation(
                    out=pAB[:, off : 512 + w],
                    in_=sAB[:, off : 512 + w],
                    func=EXP,
                    scale=0.0625,
                )
                if i >= 4 * j:  # diagonal tile: mask the leading 128-col block
                    nc.gpsimd.tensor_mul(
                        pAB[:, off : off + P], pAB[:, off : off + P], tri[:]
                    )
                    nc.gpsimd.tensor_mul(
                        pAB[:, 512 : 512 + P], pAB[:, 512 : 512 + P], tri[:]
                    )
                return pAB

            def av_mm(pair, att, s, m, j, i, pAB):
                # each att bank holds one accumulation GROUP spanning both m
                # slots: start only zeroes once (it clears the whole 2KB zero
                # region), stop only on the very last write to the bank
                cm = (m - 4 * j) * P
                off = max(0, i * P - j * 512)
                last = None
                for h in range(2):
                    lo = cm if h == 0 else 512 + cm - off
                    base = (2 * s + h) * 65
                    last = nc.tensor.matmul(
                        att[:, base : base + 65],
                        lhsT=pAB[:, lo : lo + P],
                        rhs=vt[(pair, i)][:, h, :],
                        start=(i == 0 and s == 0 and h == 0),
                        stop=(i == m and s == 1 and h == 1),
                    )
                return last

            def normalize(pair, att, s, m, an, dep=None):
                rc = rcp.tile([P, 2], F32, name="rc")
                for h in range(2):
                    base = (2 * s + h) * 65
                    r = nc.vector.reciprocal(
                        rc[:, h : h + 1], att[:, base + 64 : base + 65]
                    )
                    if dep is not None and h == 0:
                        # slot-0 values are final, but the bank's accumulation
                        # group only closes at the slot-1 stop matmul; DVE is
                        # in-order so one dep covers the whole normalize
                        add_dep_helper(r.ins, dep.ins, sync=True,
                                       reason="att group close")
                    nc.vector.tensor_scalar(
                        out=an[:, 128 * s + h * 64 : 128 * s + (h + 1) * 64],
                        in0=att[:, base : base + 64],
                        scalar1=rc[:, h : h + 1],
                        scalar2=None,
                        op0=MUL,
                    )

            def av_step(j, ms, att, ip, probs, op, pair=None):
                raise NotImplementedError

            def emit_pair(pair, jorder, budget):
                def av_step(j, ms, att, ip, probs, op):
                    for m in ms:
                        if m < ip:
                            continue
                        t, s = att[m]
                        stop = av_mm(pair, t, s, m, j, ip, probs[ip])
                        if ip == m and s == 1:
                            # group closed: normalize both slots of this bank,
                            # then one batched transpose covers the m-pair
                            an = anp.tile([P, 256], BF16, name="an")
                            normalize(pair, t, 0, m - 1, an, dep=stop)
                            normalize(pair, t, 1, m, an)
                            nc.sync.dma_start_transpose(
                                out=attnT[(pair, m // 2)][:], in_=an[:]
                            )
                            if pair == 1:
                                op(m - 1)
                                op(m)

                # scores run one step ahead of exps (lead-1) so the exp's
                # input semaphore has fired long before ACT gets there
                seq = [(j, i) for j in jorder for i in range(4 * j + 4)]
                sq = {}
                sq[seq[0]] = S_mm(pair, *seq[0])
                idx = 0
                for j in jorder:
                    last = pair == 1 and j == jorder[-1]

                    def op(m, last=last):
                        if last:
                            for _, it in emit_outproj(m, endgame=True):
                                it()
                        else:
                            bg.extend(
                                (c, t, stepno[0] + 3) for c, t in emit_outproj(m)
                            )

                    nsteps = 4 * j + 4
                    probs = {}
                    ms = list(range(4 * j, 4 * j + 4))
                    att = {}  # m -> (tile, slot)
                    pend = []  # i's whose AV is not yet emitted
                    for i in range(nsteps):
                        if idx + 1 < len(seq):
                            sq[seq[idx + 1]] = S_mm(pair, *seq[idx + 1])
                        probs[i] = S_exp(pair, j, i, sq.pop((j, i)))
                        idx += 1
                        stepno[0] += 1
                        pend.append(i)
                        # scale the dripped background work to this step's exp
                        # length so the PE never outruns ACT on short tiles
                        w = 512 - max(0, i * P - j * 512)
                        drip(max(2, budget * (512 + w) // 1024))
                        if i == 0:
                            lo = aps.tile([P, 260], F32, name="att")
                            hi = aps.tile([P, 260], F32, name="att")
                            for s, m in enumerate(ms):
                                att[m] = (lo, s) if s < 2 else (hi, s - 2)
                        if i >= 2:
                            ip = pend.pop(0)
                            av_step(j, ms, att, ip, probs, op)
                    while pend:
                        ip = pend.pop(0)
                        av_step(j, ms, att, ip, probs, op)

            # ---- schedule
            # upfront: first Q/K sweeps + first V' transposes (gate the first
            # score tile), everything else drips
            qs_up = q_sweep_items(0, 0)
            ks_up = k_sweep_items(0, 0)
            for _, it in qs_up[:-1]:
                it()
            for _, it in ks_up[:-1]:
                it()
            qs_up[-1][1]()  # emits both Q evictions
            ks_up[-1][1]()
            for _, it in vT_items(0, 0):
                it()

            order = []
            for ni in (1, 2, 3):
                order += q_sweep_items(ni, 0) + k_sweep_items(ni, 0)
                order += vT_items(0, ni)
            for ni in range(4):
                order += q_sweep_items(ni, 1) + k_sweep_items(ni, 1)
                order += vT_items(1, ni)
            bg.extend(order)

            emit_pair(0, (0, 1, 2, 3), budget=7)
            emit_pair(1, (0, 1, 2, 3), budget=5)
            while bg:
                drip(6)

    nc.compile()
    return nc


def make_in_maps(pos_encode_toks, Wq, bq, Wk, bk, W0, b0):
    x = np.asarray(pos_encode_toks, dtype=np.float32)
    Wq = np.asarray(Wq, dtype=np.float32)
    bq = np.asarray(bq, dtype=np.float32)
    Wk = np.asarray(Wk, dtype=np.float32)
    bk = np.asarray(bk, dtype=np.float32)
    W0 = np.asarray(W0, dtype=np.float32)
    in_maps = []
    for core in range(8):
        b, g = divmod(core, 4)
        hs = slice(g * HD, (g + 1) * HD)
        xt = np.ascontiguousarray(x[b].T)
        in_maps.append(
            {
                "xT": xt.astype(ml_dtypes.bfloat16),
                "x8": xt.astype(ml_dtypes.float8_e4m3),
                "Wq": np.ascontiguousarray(
                    Wq[:, hs].reshape(8, P, HD).transpose(1, 0, 2)
                ).astype(ml_dtypes.bfloat16),
                "Wk8": np.ascontiguousarray(
                    (Wk[:, hs] * 128.0).reshape(8, P, HD).transpose(1, 0, 2)
                ).astype(ml_dtypes.float8_e4m3),
                "bqt": np.ascontiguousarray(bq[hs].reshape(2, P).T),
                "bkt": np.ascontiguousarray(bk[hs].reshape(2, P).T),
                "W0": np.ascontiguousarray(W0[hs, :]).astype(ml_dtypes.bfloat16),
            }
        )
    return in_maps


def assemble(results, b0):
    out = np.zeros((2, S, D), dtype=np.float32)
    for core in range(8):
        b = core // 4
        out[b] += results[core]["out"].astype(np.float32)
    out += np.asarray(b0, dtype=np.float32)
    return out


def kernel(pos_encode_toks, Wq, bq, Wk, bk, W0, b0):
    from concourse.bass_utils import run_bass_kernel_spmd

    global _CACHED_NC
    if _CACHED_NC is None:
        _CACHED_NC = build_nc()
    in_maps = make_in_maps(pos_encode_toks, Wq, bq, Wk, bk, W0, b0)
    res = run_bass_kernel_spmd(_CACHED_NC, in_maps, core_ids=list(range(8)))
    return assemble(res.results, b0)


# revision 7
# speedup vs baseline: 1.3362x; 1.0037x over previous
"""Multi-head causal self-attention (V=Q variant) on 8 Trainium2 cores, v2.

Sharding: batch (2) x head-group (4 groups of 4 heads). Each core computes
full-sequence attention for its 4 heads (2 head-pairs) of one batch element
plus its slice of the output projection; the host sums 4 partials per batch
and adds b0.

v2 redesign vs v1:
- K projection and scores run in fp8e4m3 DoubleRow (0.5 cycles/row).
  Scores use a zeroed second k-tile so a 64-deep contraction still gets the
  DoubleRow rate. Wk is host-prescaled by 128 to keep fp8 normals; the
  1/128 and the 1/sqrt(DK) are folded into the K eviction and exp scale.
- AV is computed reversed: out[q,65] = probs_chunk^T @ [V|1], filling all
  128 output partitions per pass (the forward form only filled 65). The
  softmax denominator still falls out of the ones column; normalization is
  a per-partition divide (Pool), then attn^T is rebuilt with one PE
  transpose per (pair, m) for the output projection.
- exp is merged to one [128,1024] activation per full score tile (A and B
  halves share a 2-bank PSUM tile); ACT is the bottleneck engine.
- Evictions are spread: DVE does projections/V'/attnT, Pool does normalize
  divides and output-projection evictions.
"""

import ml_dtypes
import numpy as np

import concourse.bacc as bacc
import concourse.mybir as mybir
from concourse.tile import TileContext, add_dep_helper

P = 128
S = 2048
D = 1024
HD = 256
DK = 64
NQ = 4  # q blocks of 512
NKD = 8  # D chunks of 128
F32 = mybir.dt.float32
BF16 = mybir.dt.bfloat16
FP8 = mybir.dt.float8e4
EXP = mybir.ActivationFunctionType.Exp
DR = mybir.MatmulPerfMode.DoubleRow
MUL = mybir.AluOpType.mult
ADD = mybir.AluOpType.add
DIV = mybir.AluOpType.divide

_CACHED_NC = None


def build_nc():
    nc = bacc.Bacc("TRN2", target_bir_lowering=False, debug=False, num_devices=8)
    xT = nc.declare_dram_parameter("xT", [D, S], BF16, isOutput=False)
    x8 = nc.declare_dram_parameter("x8", [D, S], FP8, isOutput=False)
    Wq = nc.declare_dram_parameter("Wq", [P, NKD, HD], BF16, isOutput=False)
    Wk8 = nc.declare_dram_parameter("Wk8", [P, NKD, HD], FP8, isOutput=False)
    bqt = nc.declare_dram_parameter("bqt", [P, 2], F32, isOutput=False)
    bkt = nc.declare_dram_parameter("bkt", [P, 2], F32, isOutput=False)
    W0 = nc.declare_dram_parameter("W0", [HD, D], BF16, isOutput=False)
    out = nc.declare_dram_parameter("out", [S, D], BF16, isOutput=True)

    with TileContext(nc) as tc:
        with (
            tc.tile_pool(name="const", bufs=1) as const,
            tc.tile_pool(name="xt", bufs=8) as xtp,
            tc.tile_pool(name="x8p", bufs=4) as x8p,
            tc.tile_pool(name="pp", bufs=24) as ppool,
            tc.tile_pool(name="an", bufs=4) as anp,
            tc.tile_pool(name="tst", bufs=3) as tstg,
            tc.tile_pool(name="rc", bufs=4) as rcp,
            tc.tile_pool(name="ost", bufs=3) as ostp,
            tc.tile_pool(name="sps", bufs=2, space="PSUM") as sps,
            tc.tile_pool(name="aps", bufs=2, space="PSUM") as aps,
            tc.tile_pool(name="mm", bufs=2, space="PSUM") as mmp,
        ):
            # ---- PE p-state warmup first: dummy matmuls keep the PE busy
            # through the initial DMA wait so real matmuls start at full clock
            wa = const.tile([P, 256], BF16, name="wa")
            nc.gpsimd.memset(wa[:], 0.0)
            for _ in range(14):
                wps = mmp.tile([P, 256], F32, name="ps")
                nc.tensor.matmul(wps[:], lhsT=wa[:, 0:128], rhs=wa[:], start=True, stop=True)

            # ---- constants / static tiles
            identity = const.tile([P, P], BF16)
            nc.gpsimd.memset(identity[:], 0.0)
            nc.gpsimd.affine_select(
                out=identity[:],
                in_=identity[:],
                compare_op=mybir.AluOpType.not_equal,
                fill=1.0,
                base=0,
                pattern=[[-1, P]],
                channel_multiplier=1,
            )
            # triangular keep-mask [128,128]: 1.0 where q >= kv
            tri = const.tile([P, P], BF16, name="tri")
            nc.gpsimd.memset(tri[:], 1.0)
            nc.gpsimd.affine_select(
                out=tri[:],
                in_=tri[:],
                compare_op=mybir.AluOpType.is_ge,
                fill=0.0,
                base=0,
                pattern=[[1, P]],
                channel_multiplier=-1,
            )
            # ACT exp-table warmup while DMAs run
            warm = const.tile([P, 8], F32, name="warm")
            nc.gpsimd.memset(warm[:], 0.0)
            nc.scalar.activation(out=warm[:], in_=warm[:], func=EXP)

            bq_sb = const.tile([P, 2], F32)
            bk_sb = const.tile([P, 2], F32)
            w0_sb = [const.tile([P, D], BF16, name=f"w0_{p}") for p in range(2)]
            # projections: QT (bf16, feeds V' transposes), QT8/KT8 (fp8;
            # scores use a stride-0 broadcast second k-tile, doubling the
            # result -- absorbed by the exp scale)
            QT = [
                [const.tile([P, 512], BF16, name=f"qt{mi}_{ni}") for ni in range(NQ)]
                for mi in range(2)
            ]
            QT8 = [
                [const.tile([P, 1, 512], FP8, name=f"qt8{mi}_{ni}") for ni in range(NQ)]
                for mi in range(2)
            ]
            KT8 = [
                [const.tile([P, 1, 512], FP8, name=f"kt8{mi}_{ni}") for ni in range(NQ)]
                for mi in range(2)
            ]
            vt = {}
            for pair in range(2):
                for i in range(16):
                    vt[(pair, i)] = const.tile([P, 2, 65], BF16, name=f"vt{pair}_{i}")
            attnT = {}  # (pair, m-pair) -> [P, 2, P]; [:, m%2, :] = attn^T(m)
            for pair in range(2):
                for mp in range(8):
                    attnT[(pair, mp)] = const.tile([P, 2, P], BF16, name=f"at{pair}_{mp}")

            # ones column of every V' tile (Pool, pair-0 first); written once
            for key in vt:
                nc.gpsimd.memset(vt[key][:, :, 64:65], 1.0)

            # ---- DMAs, ordered so the first Q/K sweeps can start early.
            # x arrives in 512-column (q-chunk) slabs, k-major inside.
            xq = [[None, None] for _ in range(NQ)]  # [qc][lohi] -> [P, 4, 512]
            x8q = [None] * NQ  # [qc] -> [P, 8, 512] fp8
            wq_big = const.tile([P, NKD, HD], BF16, name="wqb")
            wk8_big = const.tile([P, NKD, HD], FP8, name="wkb")

            def dma_x16(qc):
                cs = slice(qc * 512, (qc + 1) * 512)
                for lohi in range(2):
                    t = xtp.tile([P, 4, 512], BF16, name="xq")
                    nc.sync.dma_start(
                        out=t[:],
                        in_=xT[lohi * 512 : (lohi + 1) * 512, cs].rearrange(
                            "(k p) s -> p k s", p=P
                        ),
                    )
                    xq[qc][lohi] = t

            def dma_x8(qc):
                cs = slice(qc * 512, (qc + 1) * 512)
                t8 = x8p.tile([P, 8, 512], FP8, name="x8q")
                nc.sync.dma_start(
                    out=t8[:], in_=x8[:, cs].rearrange("(k p) s -> p k s", p=P)
                )
                x8q[qc] = t8

            nc.sync.dma_start(out=wq_big[:], in_=Wq[:, :, :])
            dma_x16(0)
            nc.sync.dma_start(out=bq_sb[:], in_=bqt[:, :])
            nc.sync.dma_start(out=bk_sb[:], in_=bkt[:, :])
            nc.sync.dma_start(out=wk8_big[:], in_=Wk8[:, :, :])
            dma_x8(0)
            for qc in range(1, NQ):
                dma_x16(qc)
                dma_x8(qc)
            for p in range(2):
                nc.sync.dma_start(
                    out=w0_sb[p][:],
                    in_=W0[p * P : (p + 1) * P, :],
                )

            # ---- sweep emitters (as drip-able item lists)
            def q_sweep_items(ni, mi):
                ps = mmp.tile([P, 512], F32, name="ps")

                def mk(k):
                    def go():
                        nc.tensor.matmul(
                            ps[:],
                            lhsT=wq_big[:, k, mi * P : (mi + 1) * P],
                            rhs=xq[ni][k // 4][:, k % 4, :],
                            start=(k == 0),
                            stop=(k == NKD - 1),
                        )

                    return go

                def evict():
                    # two direct evictions: fp8 for scores (critical path),
                    # bf16 for the V' transposes
                    nc.vector.tensor_scalar_add(
                        QT8[mi][ni][:, 0, :], ps[:], bq_sb[:, mi : mi + 1]
                    )
                    nc.vector.tensor_scalar_add(
                        QT[mi][ni][:, :], ps[:], bq_sb[:, mi : mi + 1]
                    )

                return [(2, mk(k)) for k in range(NKD)] + [(1, evict)]

            def k_sweep_items(ni, mi):
                ps = mmp.tile([P, 512], F32, name="ps")

                def mk(kp):
                    def go():
                        nc.tensor.matmul(
                            ps[:],
                            lhsT=wk8_big[:, 2 * kp : 2 * kp + 2, mi * P : (mi + 1) * P],
                            rhs=x8q[ni][:, 2 * kp : 2 * kp + 2, :],
                            start=(kp == 0),
                            stop=(kp == 3),
                            perf_mode=DR,
                        )

                    return go

                def evict():
                    nc.vector.tensor_scalar(
                        out=KT8[mi][ni][:, 0, :],
                        in0=ps[:],
                        scalar1=1.0 / 128.0,
                        scalar2=bk_sb[:, mi : mi + 1],
                        op0=MUL,
                        op1=ADD,
                    )

                return [(2, mk(kp)) for kp in range(4)] + [(1, evict)]

            def vT_items(pair, ni):
                """one batched DMA transpose per QT tile covers 4 V' chunks"""
                state = {}

                def tp_go():
                    state["tp"] = tstg.tile([P, 4, P], BF16, name="ts")
                    nc.sync.dma_start_transpose(out=state["tp"][:], in_=QT[pair][ni][:, :])

                def cp(c):
                    def go():
                        v = vt[(pair, 4 * ni + c)]
                        src_ap = state["tp"][:, c, :].rearrange("p (h d) -> p h d", h=2)
                        nc.vector.tensor_copy(v[:, :, 0:64], src_ap)

                    return go

                return [(2, tp_go)] + [(1, cp(c)) for c in range(4)]

            # ---- drip queue: background emit-thunks (sweeps, V'T, outproj)
            # items may carry a min step number (global exp-step counter) so
            # work that waits on a fresh DMA-transpose isn't popped while its
            # input is still in flight (it would stall the in-order PE queue)
            bg = []
            stepno = [0]

            def drip(budget):
                i2 = 0
                while i2 < len(bg) and budget > 0:
                    item = bg[i2]
                    if len(item) == 3 and item[2] > stepno[0]:
                        i2 += 1
                        continue
                    bg.pop(i2)
                    item[1]()
                    budget -= item[0]

            def emit_outproj(m, endgame=False):
                # endgame (post-last-exp): evictions alternate ACT/DVE (ACT is
                # idle by then) and the out DMA goes per-half to start earlier
                state = {}

                def half(n):
                    ps = mmp.tile([P, 512], F32, name="ps")
                    for p_ in range(2):
                        nc.tensor.matmul(
                            ps[:],
                            lhsT=attnT[(p_, m // 2)][:, m % 2, :],
                            rhs=w0_sb[p_][:, n * 512 : (n + 1) * 512],
                            start=(p_ == 0),
                            stop=(p_ == 1),
                        )
                    dst = state["ot"][:, n * 512 : (n + 1) * 512]
                    if endgame and n == 0:
                        nc.scalar.copy(dst, ps[:])
                    else:
                        nc.vector.tensor_copy(dst, ps[:])

                def go0():
                    state["ot"] = ostp.tile([P, D], BF16, name="ot")
                    half(0)
                    if endgame:
                        nc.sync.dma_start(
                            out=out[m * P : (m + 1) * P, 0:512],
                            in_=state["ot"][:, 0:512],
                        )

                def go1():
                    half(1)
                    if endgame:
                        nc.sync.dma_start(
                            out=out[m * P : (m + 1) * P, 512:1024],
                            in_=state["ot"][:, 512:1024],
                        )
                    else:
                        nc.sync.dma_start(
                            out=out[m * P : (m + 1) * P, :], in_=state["ot"][:]
                        )

                return [(2, go0), (2, go1)]

            # ---- attention
            def S_mm(pair, j, i):
                """score matmuls for tile (j, i): S^T doubled via the stride-0
                second k-tile; the 2x and 1/sqrt(DK) sit in the exp scale.
                A lands at [off:512], B at [512:512+w] so one exp covers both."""
                off = max(0, i * P - j * 512)
                w = 512 - off
                kc = slice((i % 4) * P, (i % 4 + 1) * P)
                sAB = sps.tile([P, 1024], F32, name="sab")
                qs = slice(off, 512)
                for h in range(2):
                    hs = slice(h * 64, h * 64 + 64)
                    dst = sAB[:, off:512] if h == 0 else sAB[:, 512 : 512 + w]
                    nc.tensor.matmul(
                        dst,
                        lhsT=KT8[pair][i // 4][hs, :, kc].broadcast_to([64, 2, P]),
                        rhs=QT8[pair][j][hs, :, qs].broadcast_to([64, 2, w]),
                        perf_mode=DR,
                    )
                return sAB

            def S_exp(pair, j, i, sAB):
                """one exp (+ causal masks) for tile (j, i); returns probs."""
                off = max(0, i * P - j * 512)
                w = 512 - off
                pAB = ppool.tile([P, 1024], BF16, name="pab")
                nc.scalar.activation(
                    out=pAB[:, off : 512 + w],
                    in_=sAB[:, off : 512 + w],
                    func=EXP,
                    scale=0.0625,
                )
                if i >= 4 * j:  # diagonal tile: mask the leading 128-col block
                    nc.gpsimd.tensor_mul(
                        pAB[:, off : off + P], pAB[:, off : off + P], tri[:]
                    )
                    nc.gpsimd.tensor_mul(
                        pAB[:, 512 : 512 + P], pAB[:, 512 : 512 + P], tri[:]
                    )
                return pAB

            def av_mm(pair, att, s, m, j, i, pAB):
                # each att bank holds one accumulation GROUP spanning both m
                # slots: start only zeroes once (it clears the whole 2KB zero
                # region), stop only on the very last write to the bank
                cm = (m - 4 * j) * P
                off = max(0, i * P - j * 512)
                last = None
                for h in range(2):
                    lo = cm if h == 0 else 512 + cm - off
                    base = (2 * s + h) * 65
                    last = nc.tensor.matmul(
                        att[:, base : base + 65],
                        lhsT=pAB[:, lo : lo + P],
                        rhs=vt[(pair, i)][:, h, :],
                        start=(i == 0 and s == 0 and h == 0),
                        stop=(i == m and s == 1 and h == 1),
                    )
                return last

            def normalize(pair, att, s, m, an, dep=None):
                rc = rcp.tile([P, 2], F32, name="rc")
                for h in range(2):
                    base = (2 * s + h) * 65
                    r = nc.vector.reciprocal(
                        rc[:, h : h + 1], att[:, base + 64 : base + 65]
                    )
                    if dep is not None and h == 0:
                        # slot-0 values are final, but the bank's accumulation
                        # group only closes at the slot-1 stop matmul; DVE is
                        # in-order so one dep covers the whole normalize
                        add_dep_helper(r.ins, dep.ins, sync=True,
                                       reason="att group close")
                    nc.vector.tensor_scalar(
                        out=an[:, 128 * s + h * 64 : 128 * s + (h + 1) * 64],
                        in0=att[:, base : base + 64],
                        scalar1=rc[:, h : h + 1],
                        scalar2=None,
                        op0=MUL,
                    )

            def av_step(j, ms, att, ip, probs, op, pair=None):
                raise NotImplementedError

            def emit_pair(pair, jorder, budget):
                def av_step(j, ms, att, ip, probs, op):
                    for m in ms:
                        if m < ip:
                            continue
                        t, s = att[m]
                        stop = av_mm(pair, t, s, m, j, ip, probs[ip])
                        if ip == m and s == 1:
                            # group closed: normalize both slots of this bank,
                            # then one batched transpose covers the m-pair
                            an = anp.tile([P, 256], BF16, name="an")
                            normalize(pair, t, 0, m - 1, an, dep=stop)
                            normalize(pair, t, 1, m, an)
                            nc.sync.dma_start_transpose(
                                out=attnT[(pair, m // 2)][:], in_=an[:]
                            )
                            if pair == 1:
                                op(m - 1)
                                op(m)

                # scores run one step ahead of exps (lead-1) so the exp's
                # input semaphore has fired long before ACT gets there
                seq = [(j, i) for j in jorder for i in range(4 * j + 4)]
                sq = {}
                sq[seq[0]] = S_mm(pair, *seq[0])
                idx = 0
                for j in jorder:
                    last = pair == 1 and j == jorder[-1]

                    def op(m, last=last):
                        if last:
                            for _, it in emit_outproj(m, endgame=True):
                                it()
                        else:
                            bg.extend(
                                (c, t, stepno[0] + 3) for c, t in emit_outproj(m)
                            )

                    nsteps = 4 * j + 4
                    probs = {}
                    ms = list(range(4 * j, 4 * j + 4))
                    att = {}  # m -> (tile, slot)
                    pend = []  # i's whose AV is not yet emitted
                    for i in range(nsteps):
                        if idx + 1 < len(seq):
                            sq[seq[idx + 1]] = S_mm(pair, *seq[idx + 1])
                        probs[i] = S_exp(pair, j, i, sq.pop((j, i)))
                        idx += 1
                        stepno[0] += 1
                        pend.append(i)
                        # scale the dripped background work to this step's exp
                        # length so the PE never outruns ACT on short tiles
                        w = 512 - max(0, i * P - j * 512)
                        drip(max(2, budget * (512 + w) // 1024))
                        if i == 0:
                            lo = aps.tile([P, 260], F32, name="att")
                            hi = aps.tile([P, 260], F32, name="att")
                            for s, m in enumerate(ms):
                                att[m] = (lo, s) if s < 2 else (hi, s - 2)
                        if i >= 3:
                            ip = pend.pop(0)
                            av_step(j, ms, att, ip, probs, op)
                    while pend:
                        ip = pend.pop(0)
                        av_step(j, ms, att, ip, probs, op)

            # ---- schedule
            # upfront: first Q/K sweeps + first V' transposes (gate the first
            # score tile), everything else drips
            qs_up = q_sweep_items(0, 0)
            ks_up = k_sweep_items(0, 0)
            for _, it in qs_up[0:4]:
                it()
            for _, it in ks_up[:-1]:
                it()
            for _, it in qs_up[4:8]:
                it()
            ks_up[-1][1]()  # K eviction first (its data lands earlier)
            qs_up[-1][1]()  # then both Q evictions

            for _, it in vT_items(0, 0):
                it()

            order = []
            for ni in (1, 2, 3):
                order += q_sweep_items(ni, 0) + k_sweep_items(ni, 0)
                order += vT_items(0, ni)
            for ni in range(4):
                order += q_sweep_items(ni, 1) + k_sweep_items(ni, 1)
                order += vT_items(1, ni)
            bg.extend(order)

            emit_pair(0, (0, 1, 2, 3), budget=7)
            emit_pair(1, (0, 1, 2, 3), budget=5)
            while bg:
                drip(6)

    nc.compile()
    return nc


def make_in_maps(pos_encode_toks, Wq, bq, Wk, bk, W0, b0):
    x = np.asarray(pos_encode_toks, dtype=np.float32)
    Wq = np.asarray(Wq, dtype=np.float32)
    bq = np.asarray(bq, dtype=np.float32)
    Wk = np.asarray(Wk, dtype=np.float32)
    bk = np.asarray(bk, dtype=np.float32)
    W0 = np.asarray(W0, dtype=np.float32)
    in_maps = []
    for core in range(8):
        b, g = divmod(core, 4)
        hs = slice(g * HD, (g + 1) * HD)
        xt = np.ascontiguousarray(x[b].T)
        in_maps.append(
            {
                "xT": xt.astype(ml_dtypes.bfloat16),
                "x8": xt.astype(ml_dtypes.float8_e4m3),
                "Wq": np.ascontiguousarray(
                    Wq[:, hs].reshape(8, P, HD).transpose(1, 0, 2)
                ).astype(ml_dtypes.bfloat16),
                "Wk8": np.ascontiguousarray(
                    (Wk[:, hs] * 128.0).reshape(8, P, HD).transpose(1, 0, 2)
                ).astype(ml_dtypes.float8_e4m3),
                "bqt": np.ascontiguousarray(bq[hs].reshape(2, P).T),
                "bkt": np.ascontiguousarray(bk[hs].reshape(2, P).T),
                "W0": np.ascontiguousarray(W0[hs, :]).astype(ml_dtypes.bfloat16),
            }
        )
    return in_maps


def assemble(results, b0):
    out = np.zeros((2, S, D), dtype=np.float32)
    for core in range(8):
        b = core // 4
        out[b] += results[core]["out"].astype(np.float32)
    out += np.asarray(b0, dtype=np.float32)
    return out


def kernel(pos_encode_toks, Wq, bq, Wk, bk, W0, b0):
    from concourse.bass_utils import run_bass_kernel_spmd

    global _CACHED_NC
    if _CACHED_NC is None:
        _CACHED_NC = build_nc()
    in_maps = make_in_maps(pos_encode_toks, Wq, bq, Wk, bk, W0, b0)
    res = run_bass_kernel_spmd(_CACHED_NC, in_maps, core_ids=list(range(8)))
    return assemble(res.results, b0)
